# revision 1
# baseline (speedup 1.0000x reference)
"""Trainium2 Bass kernel for nn_ChromaticResonance.

Reference computation (per batch row, complex wave w of dim D=512):
  7 depths of: y = w@(C+H1) [+ w for d>0, folded as +I into the matrix]
               + 0.25*|w@H2|^2                       (real only)
               + (1/9)*|w@H3|^2 * (w@H3)
               + 0.04*(w@H5)^5 * |w@H5|^-4.8
       nl = tanh(y*scale + bias)  (componentwise re/im)
       w' = exp(-damping*d) * nl
  out = sum_d softmax-ish-weighted w'_d

Implementation strategy (8 cores, pure data parallel over batch):
  - Transposed layout [D, B]: D maps to partitions (4 k-tiles of 128),
    batch maps to the free dim, processed in chunks of NB=512 columns.
  - Complex packed tiles [128, 2*NB]: re in cols :NB, im in cols NB:.
    Matmuls run in fp32r (~2.7x fp32 rate, ~1e-4 rel) in two PSUM waves
    of 4 banks each (H2+H3, then W1+H5) so the PE pipelines across m.
  - The whole 7-depth recurrence runs on-chip per chunk; HBM traffic is
    input once + output once (memory target regime).
  - z^5 * r2^-2.4 is computed as Exp(scale*X + bias) * z*(z^2)^2 where X
    is a bit-trick log2 (bitcast + quadratic mantissa correction) — keeps
    every ACT function in one table set (no 2.7us table switches).
  - h5 internals run in bf16 (term weight 0.04 tolerates it); the acc
    chain, tanh and output accumulation stay fp32.
"""

import numpy as np

import concourse.bass as bass
import concourse.mybir as mybir
import concourse.tile as tile
from concourse import bass_utils
from concourse.bacc import Bacc

F32 = mybir.dt.float32
F32R = mybir.dt.float32r
F16 = mybir.dt.float16
BF16 = mybir.dt.bfloat16
I32 = mybir.dt.int32
AF = mybir.ActivationFunctionType
OP = mybir.AluOpType

B, D, DEPTH = 32768, 512, 7
N_CORES = 8
BS = B // N_CORES          # batch rows per core
NB = 512                   # batch columns per chunk
KT = D // 128              # 4 partition tiles of the D dim
LN004 = float(np.log(0.04))
# fast-log2 (bitcast) constants: log2(x) = 2^-23*i - 127 + sigma(f),
# sigma(f) ~ C1*f + C2*f^2 minimax-fit (max err 0.0064)
LOG_C1 = 0.3560081305460369
LOG_C2S = -4.3151684254e-08          # C2 * 2^-23
EXP_SCALE = float(-2.4 * np.log(2.0) * 2.0 ** -23)
EXP_BIAS = float(127 * 2.4 * np.log(2.0) + np.log(0.04))


def _dup2(t):
    """Broadcast a [128, NB] AP to [128, 2, NB] (each column read twice)."""
    ap = t.ap
    return bass.AP(tensor=t.tensor, offset=t.offset, ap=[ap[0], [0, 2], ap[1]])


def _as3(t):
    """View a [128, 2*NB] AP as [128, 2, NB]."""
    return t.rearrange("p (two n) -> p two n", two=2)


def build_program(n_chunks=BS // NB, nb=NB):
    nc = Bacc()
    bcols = n_chunks * nb

    wre = nc.dram_tensor("wre", [D, bcols], F32R, kind="ExternalInput")
    wim = nc.dram_tensor("wim", [D, bcols], F32R, kind="ExternalInput")
    wmat = nc.dram_tensor("wmat", [5, D, D], F32R, kind="ExternalInput")
    consts = nc.dram_tensor("consts", [D, 16], F32, kind="ExternalInput")
    ore = nc.dram_tensor("ore", [D, bcols], F32, kind="ExternalOutput")
    oim = nc.dram_tensor("oim", [D, bcols], F32, kind="ExternalOutput")

    H = slice(0, nb)       # real half of a packed tile
    I = slice(nb, 2 * nb)  # imag half

    with tile.TileContext(nc) as tc:
        with (
            tc.tile_pool(name="wpool", bufs=1) as wpool,
            tc.tile_pool(name="chpool", bufs=1) as chpool,
            tc.tile_pool(name="opool", bufs=1) as opool,
            tc.tile_pool(name="ppool", bufs=1, space="PSUM") as ppool,
            tc.tile_pool(name="s2", bufs=2) as s2,      # [128, 2nb] scratch
            tc.tile_pool(name="s1", bufs=2) as s1,      # [128, nb] scratch
        ):
            # ---- load weights + constants (once) ----
            wt = []
            for mi in range(5):
                w = wpool.tile([128, KT, D], F32R, name=f"wt{mi}", tag=f"wt{mi}")
                for k in range(KT):
                    nc.sync.dma_start(out=w[:, k, :], in_=wmat[mi, k * 128:(k + 1) * 128, :])
                wt.append(w)
            cons = []
            for m in range(KT):
                c = wpool.tile([128, 16], F32, name=f"cons{m}", tag=f"cons{m}")
                nc.sync.dma_start(out=c, in_=consts[m * 128:(m + 1) * 128, :])
                cons.append(c)

            for ci in range(n_chunks):
                c0 = ci * nb
                ch = []
                for k in range(KT):
                    t = chpool.tile([128, 2 * nb], F32R, name=f"cha{k}", tag=f"cha{k}")
                    nc.sync.dma_start(out=t[:, H], in_=wre[k * 128:(k + 1) * 128, c0:c0 + nb])
                    nc.sync.dma_start(out=t[:, I], in_=wim[k * 128:(k + 1) * 128, c0:c0 + nb])
                    ch.append(t)
                out_t = [opool.tile([128, 2 * nb], F32, name=f"out{m}", tag=f"out{m}")
                         for m in range(KT)]

                for d in range(DEPTH):
                    w1 = wt[0] if d == 0 else wt[1]
                    ch_next = None
                    if d < DEPTH - 1:
                        pong = "b" if d % 2 == 0 else "a"
                        ch_next = [chpool.tile([128, 2 * nb], F32R,
                                               name=f"ch{pong}{m}", tag=f"ch{pong}{m}")
                                   for m in range(KT)]

                    for m in range(KT):
                        msl = slice(m * 128, (m + 1) * 128)
                        # ---- wave A: H2, H3 matmuls (2 banks each) ----
                        psA = [ppool.tile([128, 2 * nb], F32, name=f"psA{j}", tag=f"psA{j}")
                               for j in range(2)]
                        for j, lw in enumerate((wt[2], wt[3])):
                            for k in range(KT):
                                for hs in (H, I):
                                    nc.tensor.matmul(psA[j][:, hs], lw[:, k, msl],
                                                     ch[k][:, hs],
                                                     start=(k == 0), stop=(k == KT - 1))
                        sq2 = s2.tile([128, 2 * nb], F32, name="sq2", tag="sq2")
                        sq3 = s2.tile([128, 2 * nb], F32, name="sq3", tag="sq3")
                        nc.scalar.activation(sq2, psA[0][:, :], AF.Square)
                        nc.scalar.activation(sq3, psA[1][:, :], AF.Square)
                        r2a = s1.tile([128, nb], F32, name="r2a", tag="r2a")
                        r2b = s1.tile([128, nb], F32, name="r2b", tag="r2b")
                        nc.gpsimd.tensor_tensor(r2a, sq2[:, H], sq2[:, I], op=OP.add)
                        nc.gpsimd.tensor_tensor(r2b, sq3[:, H], sq3[:, I], op=OP.add)
                        acc = s2.tile([128, 2 * nb], F32, name="acc", tag="acc", bufs=3)
                        nc.vector.scalar_tensor_tensor(
                            _as3(acc), _dup2(r2b[:, :]), 1.0 / 9.0, _as3(psA[1][:, :]),
                            op0=OP.mult, op1=OP.mult)

                        # ---- wave B: W1, H5 matmuls ----
                        psB = [ppool.tile([128, 2 * nb], F32, name=f"psB{j}", tag=f"psB{j}")
                               for j in range(2)]
                        for j, lw in enumerate((w1, wt[4])):
                            for k in range(KT):
                                for hs in (H, I):
                                    nc.tensor.matmul(psB[j][:, hs], lw[:, k, msl],
                                                     ch[k][:, hs],
                                                     start=(k == 0), stop=(k == KT - 1))
                        sq5 = s2.tile([128, 2 * nb], BF16, name="sq5", tag="sq5")
                        d5s = s2.tile([128, 2 * nb], BF16, name="d5s", tag="d5s")
                        nc.scalar.activation(sq5, psB[1][:, :], AF.Square)
                        nc.scalar.copy(d5s, psB[1][:, :])
                        nc.vector.tensor_tensor(acc[:, :], acc[:, :], psB[0][:, :], op=OP.add)
                        nc.vector.scalar_tensor_tensor(acc[:, H], r2a[:, :], 0.25,
                                                       acc[:, H], op0=OP.mult, op1=OP.add)
                        r2d = s1.tile([128, nb], F32, name="r2d", tag="r2d")
                        c2r = s1.tile([128, nb], BF16, name="c2r", tag="c2r")
                        nc.gpsimd.tensor_tensor(r2d, sq5[:, H], sq5[:, I], op=OP.add)
                        nc.vector.tensor_tensor(c2r, sq5[:, H], sq5[:, I], op=OP.subtract)

                        # ---- t5 = 0.04 * r2d^-2.4 via bit-trick log2 + Exp ----
                        i_f = s1.tile([128, nb], F32, name="i_f", tag="i_f")
                        mqi = s1.tile([128, nb], I32, name="mqi", tag="mqi")
                        mq = s1.tile([128, nb], F32, name="mq", tag="mq")
                        nc.scalar.copy(i_f, r2d[:, :].bitcast(I32))
                        nc.vector.tensor_scalar(mqi, r2d[:, :].bitcast(I32), 0x007FFFFF,
                                                None, op0=OP.bitwise_and)
                        nc.scalar.copy(mq, mqi[:, :])
                        ff = s1.tile([128, nb], F32, name="ff", tag="ff")
                        nc.scalar.activation(ff, mq, AF.Square)
                        ux = s1.tile([128, nb], F32, name="ux", tag="ux")
                        nc.vector.scalar_tensor_tensor(ux, mq, LOG_C1, i_f,
                                                       op0=OP.mult, op1=OP.add)
                        nc.vector.scalar_tensor_tensor(ux, ff, LOG_C2S, ux,
                                                       op0=OP.mult, op1=OP.add)
                        t5 = s1.tile([128, nb], BF16, name="t5", tag="t5")
                        nc.scalar.activation(t5, ux, AF.Exp, scale=EXP_SCALE,
                                             bias=cons[m][:, 6:7])

                        # ---- z^5 = z*(z^2)^2 (bf16 internals; h5 budget is 25x) ----
                        # c2i here holds 2*(2*dr*di); Square(0.5*x) and c2r*x recover
                        # the true c2i^2 and 2*c2r*c2i without extra scale ops.
                        c2i = s1.tile([128, nb], BF16, name="c2i", tag="c2i")
                        nc.vector.scalar_tensor_tensor(c2i, d5s[:, H], 4.0, d5s[:, I],
                                                       op0=OP.mult, op1=OP.mult)
                        sq2r = s1.tile([128, nb], BF16, name="sq2r", tag="sq2r")
                        sqc2i = s1.tile([128, nb], BF16, name="sqc2i", tag="sqc2i")
                        nc.scalar.activation(sq2r, c2r, AF.Square)
                        nc.scalar.activation(sqc2i, c2i, AF.Square, scale=0.5)
                        c4r = s1.tile([128, nb], BF16, name="c4r", tag="c4r")
                        c4i = s1.tile([128, nb], BF16, name="c4i", tag="c4i")
                        nc.gpsimd.tensor_tensor(c4r, sq2r, sqc2i, op=OP.subtract)
                        nc.vector.tensor_tensor(c4i, c2r, c2i, op=OP.mult)
                        q1 = s1.tile([128, nb], BF16, name="q1", tag="q1")
                        q2 = s1.tile([128, nb], BF16, name="q2", tag="q2")
                        q3 = s1.tile([128, nb], BF16, name="q3", tag="q3")
                        q4 = s1.tile([128, nb], BF16, name="q4", tag="q4")
                        p5 = s2.tile([128, 2 * nb], BF16, name="p5", tag="p5")
                        nc.gpsimd.tensor_tensor(q1, c4r, d5s[:, H], op=OP.mult)
                        nc.gpsimd.tensor_tensor(q2, c4i, d5s[:, I], op=OP.mult)
                        nc.vector.tensor_tensor(p5[:, H], q1, q2, op=OP.subtract)
                        nc.vector.tensor_tensor(q3, c4r, d5s[:, I], op=OP.mult)
                        nc.vector.tensor_tensor(q4, c4i, d5s[:, H], op=OP.mult)
                        nc.gpsimd.tensor_tensor(p5[:, I], q3, q4, op=OP.add)
                        h5 = s2.tile([128, 2 * nb], BF16, name="h5", tag="h5")
                        nc.gpsimd.tensor_tensor(_as3(h5), _dup2(t5[:, :]), _as3(p5[:, :]),
                                                op=OP.mult)
                        nc.vector.tensor_tensor(acc[:, :], acc[:, :], h5[:, :], op=OP.add)

                        nl = s2.tile([128, 2 * nb], F32, name="nl", tag="nl", bufs=3)
                        nc.scalar.activation(nl, acc[:, :], AF.Tanh,
                                             scale=cons[m][:, 14:15], bias=cons[m][:, 15:16])
                        if ch_next is not None:
                            nc.scalar.mul(ch_next[m][:, :], nl[:, :], cons[m][:, d:d + 1])
                        if d == 0:
                            nc.vector.tensor_scalar_mul(out_t[m][:, :], nl[:, :],
                                                        cons[m][:, 7 + d:8 + d])
                        else:
                            nc.vector.scalar_tensor_tensor(
                                out_t[m][:, :], nl[:, :], cons[m][:, 7 + d:8 + d],
                                out_t[m][:, :], op0=OP.mult, op1=OP.add)
                    if ch_next is not None:
                        ch = ch_next

                for m in range(KT):
                    nc.sync.dma_start(out=ore[m * 128:(m + 1) * 128, c0:c0 + nb],
                                      in_=out_t[m][:, H])
                    nc.sync.dma_start(out=oim[m * 128:(m + 1) * 128, c0:c0 + nb],
                                      in_=out_t[m][:, I])
    nc.finalize()
    return nc


def host_prep(coupling_matrix, harmonic_1, harmonic_2, harmonic_3, harmonic_5,
              mixing_scale, mixing_bias):
    damping = (0.1 / (1.0 + np.exp(np.linspace(0.0, 3.0, D)))).astype(np.float32)
    w = np.exp(-np.linspace(0.0, 2.0, DEPTH))
    w = (w / w.sum()).astype(np.float32)
    fd = np.stack([np.exp(-damping.astype(np.float64) * float(dd))
                   for dd in range(DEPTH)]).astype(np.float32)      # [7, D]
    wf = (w[:, None] * fd).astype(np.float32)                        # [7, D]
    w1_0 = (coupling_matrix + harmonic_1).astype(np.float32)
    w1_r = (w1_0 + np.eye(D, dtype=np.float32)).astype(np.float32)
    wmat = np.ascontiguousarray(
        np.stack([w1_0, w1_r, harmonic_2, harmonic_3, harmonic_5]).astype(np.float32))
    consts = np.zeros((D, 16), np.float32)
    consts[:, 0:DEPTH] = fd.T
    consts[:, 7:7 + DEPTH] = wf.T
    consts[:, 6] = EXP_BIAS  # fd_6 never read (no chamber after last depth)
    consts[:, 14] = mixing_scale.astype(np.float32)
    consts[:, 15] = mixing_bias.astype(np.float32)
    return wmat, consts


_NC_CACHE = {}


def _get_nc(n_chunks, nb):
    key = (n_chunks, nb)
    if key not in _NC_CACHE:
        _NC_CACHE[key] = build_program(n_chunks, nb)
    return _NC_CACHE[key]


def kernel(wave_real, wave_imag, coupling_matrix, harmonic_1, harmonic_2,
           harmonic_3, harmonic_5, mixing_scale, mixing_bias):
    wmat, consts = host_prep(coupling_matrix, harmonic_1, harmonic_2,
                             harmonic_3, harmonic_5, mixing_scale, mixing_bias)
    wreT = np.ascontiguousarray(np.asarray(wave_real, np.float32).T)  # [D, B]
    wimT = np.ascontiguousarray(np.asarray(wave_imag, np.float32).T)

    nc = _get_nc(BS // NB, NB)
    in_maps = []
    for c in range(N_CORES):
        sl = slice(c * BS, (c + 1) * BS)
        in_maps.append({
            "wre": np.ascontiguousarray(wreT[:, sl]),
            "wim": np.ascontiguousarray(wimT[:, sl]),
            "wmat": wmat,
            "consts": consts,
        })
    res = bass_utils.run_bass_kernel_spmd(nc, in_maps, core_ids=list(range(N_CORES)))
    out = np.empty((2, B, D), np.float32)
    for c in range(N_CORES):
        sl = slice(c * BS, (c + 1) * BS)
        out[0, sl, :] = res.results[c]["ore"].T
        out[1, sl, :] = res.results[c]["oim"].T
    return out



# revision 4
# speedup vs baseline: 1.2119x; 1.2119x over previous
"""Trainium2 Bass kernel for nn_ChromaticResonance (v2: all-bf16 pipeline).

Reference computation (per batch row, complex wave w of dim D=512):
  7 depths of: y = w@(C+H1) [+ w for d>0, folded as +I into the matrix]
               + 0.25*|w@H2|^2                       (real only)
               + (1/9)*|w@H3|^2 * (w@H3)
               + 0.04*(w@H5)^5 * |w@H5|^-4.8
       nl = tanh(y*scale + bias)  (componentwise re/im)
       w' = exp(-damping*d) * nl
  out = sum_d w_d * w'_d

v2 strategy (8 cores, data parallel over batch; transposed [D, B] layout):
  - bf16 matmuls (1 cyc/row on PE, same as fp32r, but half the SBUF/LDW
    traffic); psum f32; nb=1024 batch cols per chunk, 4 chunks per core.
  - Single-matrix psum waves ([128, 2048] f32 = 4 banks, 2-slot pingpong);
    the ACT engine is the sole psum drainer (Square/Copy -> bf16 sbuf).
  - ALL pointwise in bf16 on SBUF: DVE tensor_tensor runs 2 elem/cyc,
    tensor_scalar 4 elem/cyc; six 1-unit ops parked on Pool. Scale factors
    (0.25 for H2, 1/9 for H3) are folded into the weights host-side.
  - t5 = 0.04*r2^-2.4 via bf16-bit fastpow: ONE tensor_scalar on the
    uint16 bit pattern (t5bits = -2.4*bits + K16), bitcast back. Max err
    ~11% on a term worth 4% of acc -> ~0.3% output. Total measured rel
    err of this pipeline vs f64 reference: ~0.7% (gate 2e-2).
"""

import numpy as np
import ml_dtypes

import concourse.bass as bass
import concourse.mybir as mybir
import concourse.tile as tile
from concourse import bass_utils
from concourse.bacc import Bacc

F32 = mybir.dt.float32
BF16 = mybir.dt.bfloat16
U16 = mybir.dt.uint16
I16 = mybir.dt.int16
AF = mybir.ActivationFunctionType
OP = mybir.AluOpType

B, D, DEPTH = 32768, 512, 7
N_CORES = 8
BS = B // N_CORES          # batch rows per core
NB = 1024                  # batch columns per chunk
KT = D // 128              # 4 partition tiles of the D dim
# bf16-bit fastpow: bits(0.04*x^-2.4) ~= -2.4*bits(x) + K16
K16 = 54657.5
S3 = float(9.0 ** (-1.0 / 3.0))   # folded into H3 so |h3'|^2*h3' = |h3|^2*h3/9


def _dup2(t, nb=NB):
    """Broadcast a [128, nb] AP to [128, 2, nb] (each column read twice)."""
    ap = t.ap
    return bass.AP(tensor=t.tensor, offset=t.offset, ap=[ap[0], [0, 2], ap[1]])


def _as3(t, nb=NB):
    """View a [128, 2*nb] AP as [128, 2, nb]."""
    return t.rearrange("p (two n) -> p two n", two=2)


def build_program(n_chunks=BS // NB, nb=NB):
    nc = Bacc()
    bcols = n_chunks * nb

    wre = nc.dram_tensor("wre", [D, bcols], BF16, kind="ExternalInput")
    wim = nc.dram_tensor("wim", [D, bcols], BF16, kind="ExternalInput")
    wmat = nc.dram_tensor("wmat", [5, D, D], BF16, kind="ExternalInput")
    consts = nc.dram_tensor("consts", [D, 16], F32, kind="ExternalInput")
    ore = nc.dram_tensor("ore", [D, bcols], BF16, kind="ExternalOutput")
    oim = nc.dram_tensor("oim", [D, bcols], BF16, kind="ExternalOutput")

    H = slice(0, nb)       # real half of a packed tile
    I = slice(nb, 2 * nb)  # imag half
    wout = np.exp(-np.linspace(0.0, 2.0, DEPTH))
    wout = [float(x) for x in (wout / wout.sum())]

    with tile.TileContext(nc) as tc:
        with (
            tc.tile_pool(name="wpool", bufs=1) as wpool,
            tc.tile_pool(name="chpool", bufs=1) as chpool,
            tc.tile_pool(name="opool", bufs=1) as opool,
            tc.tile_pool(name="ppool", bufs=1, space="PSUM") as ppool,
            tc.tile_pool(name="s2", bufs=2) as s2,      # [128, 2nb] scratch
            tc.tile_pool(name="s1", bufs=2) as s1,      # [128, nb] scratch
        ):
            # ---- load weights + constants (once) ----
            wt = []
            for mi in range(5):
                w = wpool.tile([128, KT, D], BF16, name=f"wt{mi}", tag=f"wt{mi}")
                for k in range(KT):
                    nc.sync.dma_start(out=w[:, k, :], in_=wmat[mi, k * 128:(k + 1) * 128, :])
                wt.append(w)
            cons = []
            for m in range(KT):
                c = wpool.tile([128, 16], F32, name=f"cons{m}", tag=f"cons{m}")
                nc.sync.dma_start(out=c, in_=consts[m * 128:(m + 1) * 128, :])
                cons.append(c)

            for ci in range(n_chunks):
                c0 = ci * nb
                ch = []
                for k in range(KT):
                    t = chpool.tile([128, 2 * nb], BF16, name=f"cha{k}", tag=f"cha{k}")
                    nc.sync.dma_start(out=t[:, H], in_=wre[k * 128:(k + 1) * 128, c0:c0 + nb])
                    nc.sync.dma_start(out=t[:, I], in_=wim[k * 128:(k + 1) * 128, c0:c0 + nb])
                    ch.append(t)
                out_t = [opool.tile([128, 2 * nb], BF16, name=f"out{m}", tag=f"out{m}")
                         for m in range(KT)]

                for d in range(DEPTH):
                    w1 = wt[0] if d == 0 else wt[1]
                    ch_next = None
                    if d < DEPTH - 1:
                        pong = "b" if d % 2 == 0 else "a"
                        ch_next = [chpool.tile([128, 2 * nb], BF16,
                                               name=f"ch{pong}{m}", tag=f"ch{pong}{m}")
                                   for m in range(KT)]

                    for m in range(KT):
                        msl = slice(m * 128, (m + 1) * 128)

                        def wave(lw, ps):
                            for j in range(2 * nb // 512):
                                js = slice(j * 512, (j + 1) * 512)
                                for k in range(KT):
                                    nc.tensor.matmul(ps[:, js], lw[:, k, msl],
                                                     ch[k][:, js],
                                                     start=(k == 0), stop=(k == KT - 1))

                        # ---- matmul waves + ACT psum drains ----
                        ps_h2 = ppool.tile([128, 2 * nb], F32, name="psA", tag="psA")
                        wave(wt[2], ps_h2)
                        ps_h3 = ppool.tile([128, 2 * nb], F32, name="psB", tag="psB")
                        wave(wt[3], ps_h3)
                        sq2h = s1.tile([128, nb], BF16, name="sq2h", tag="sq2h")
                        sq2i = s1.tile([128, nb], BF16, name="sq2i", tag="sq2i")
                        nc.scalar.activation(sq2h, ps_h2[:, H], AF.Square)
                        nc.scalar.activation(sq2i, ps_h2[:, I], AF.Square)
                        ps_w1 = ppool.tile([128, 2 * nb], F32, name="psA", tag="psA")
                        wave(w1, ps_w1)
                        d3s = s2.tile([128, 2 * nb], BF16, name="d3s", tag="d3s")
                        sq3 = s2.tile([128, 2 * nb], BF16, name="sq3", tag="sq3")
                        nc.scalar.copy(d3s, ps_h3[:, :])
                        nc.scalar.activation(sq3, ps_h3[:, :], AF.Square)
                        ps_h5 = ppool.tile([128, 2 * nb], F32, name="psB", tag="psB")
                        wave(wt[4], ps_h5)
                        acc = s2.tile([128, 2 * nb], BF16, name="acc", tag="acc")
                        nc.scalar.copy(acc, ps_w1[:, :])
                        d5s = s2.tile([128, 2 * nb], BF16, name="d5s", tag="d5s")
                        nc.scalar.copy(d5s, ps_h5[:, :])

                        # ---- h2 + h3 folds (weights pre-scaled host-side) ----
                        r2a = s1.tile([128, nb], BF16, name="r2a", tag="r2a")
                        nc.gpsimd.tensor_tensor(r2a, sq2h, sq2i, op=OP.add)
                        r2b = s1.tile([128, nb], BF16, name="r2b", tag="r2b")
                        nc.vector.tensor_tensor(r2b, sq3[:, H], sq3[:, I], op=OP.add)
                        ht = s2.tile([128, 2 * nb], BF16, name="ht", tag="ht", bufs=1)
                        nc.vector.tensor_tensor(_as3(ht), _dup2(r2b[:, :]), _as3(d3s),
                                                op=OP.mult)
                        nc.vector.tensor_tensor(acc[:, :], acc[:, :], ht[:, :], op=OP.add)
                        nc.vector.tensor_tensor(acc[:, H], acc[:, H], r2a, op=OP.add)

                        # ---- h5: t5 = 0.04*r2^-2.4 via bf16-bit fastpow ----
                        sq5 = s2.tile([128, 2 * nb], BF16, name="sq5", tag="sq5")
                        nc.vector.tensor_tensor(sq5[:, :], d5s[:, :], d5s[:, :], op=OP.mult)
                        r2d = s1.tile([128, nb], BF16, name="r2d", tag="r2d")
                        nc.vector.tensor_tensor(r2d, sq5[:, H], sq5[:, I], op=OP.add)
                        t5w = s1.tile([128, nb], I16, name="t5w", tag="t5w")
                        nc.vector.tensor_scalar(t5w, r2d[:, :].bitcast(U16), -2.4, K16,
                                                op0=OP.mult, op1=OP.add)
                        t5 = t5w[:, :].bitcast(BF16)
                        c2r = s1.tile([128, nb], BF16, name="c2r", tag="c2r")
                        nc.vector.tensor_tensor(c2r, sq5[:, H], sq5[:, I], op=OP.subtract)
                        e5 = s1.tile([128, nb], BF16, name="e5", tag="e5")
                        nc.vector.tensor_tensor(e5, d5s[:, H], d5s[:, I], op=OP.mult)
                        e2 = s1.tile([128, nb], BF16, name="e2", tag="e2", bufs=1)
                        nc.vector.tensor_scalar(e2, e5, 2.0, None, op0=OP.mult)
                        c22 = s1.tile([128, nb], BF16, name="c22", tag="c22", bufs=1)
                        nc.vector.tensor_scalar(c22, c2r, 2.0, None, op0=OP.mult)
                        ee4 = s1.tile([128, nb], BF16, name="ee4", tag="ee4", bufs=1)
                        nc.gpsimd.tensor_tensor(ee4, e2, e2, op=OP.mult)
                        sq2r5 = s1.tile([128, nb], BF16, name="sq2r5", tag="sq2r5", bufs=1)
                        nc.gpsimd.tensor_tensor(sq2r5, c2r, c2r, op=OP.mult)
                        mc4r = s1.tile([128, nb], BF16, name="mc4r", tag="mc4r", bufs=1)
                        nc.vector.tensor_tensor(mc4r, ee4, sq2r5, op=OP.subtract)
                        c4i4 = s1.tile([128, nb], BF16, name="c4i4", tag="c4i4", bufs=1)
                        nc.gpsimd.tensor_tensor(c4i4, e2, c22, op=OP.mult)
                        td5 = s2.tile([128, 2 * nb], BF16, name="td5", tag="td5", bufs=1)
                        nc.vector.tensor_tensor(_as3(td5), _dup2(t5), _as3(d5s), op=OP.mult)
                        # p5hn = -(p5 real); p5i = p5 imag (mc4r = -c4r*... sign flip)
                        q1 = s1.tile([128, nb], BF16, name="q1", tag="q1", bufs=1)
                        q2 = s1.tile([128, nb], BF16, name="q2", tag="q2", bufs=1)
                        q3 = s1.tile([128, nb], BF16, name="q3", tag="q3", bufs=1)
                        q4 = s1.tile([128, nb], BF16, name="q4", tag="q4", bufs=1)
                        nc.vector.tensor_tensor(q1, mc4r, td5[:, H], op=OP.mult)
                        nc.gpsimd.tensor_tensor(q2, c4i4, td5[:, I], op=OP.mult)
                        p5hn = s1.tile([128, nb], BF16, name="p5hn", tag="p5hn", bufs=1)
                        nc.vector.tensor_tensor(p5hn, q1, q2, op=OP.add)
                        nc.gpsimd.tensor_tensor(q3, c4i4, td5[:, H], op=OP.mult)
                        nc.vector.tensor_tensor(q4, mc4r, td5[:, I], op=OP.mult)
                        p5i = s1.tile([128, nb], BF16, name="p5i", tag="p5i", bufs=1)
                        nc.vector.tensor_tensor(p5i, q3, q4, op=OP.subtract)
                        nc.vector.tensor_tensor(acc[:, H], acc[:, H], p5hn, op=OP.subtract)
                        nc.vector.tensor_tensor(acc[:, I], acc[:, I], p5i, op=OP.add)

                        # ---- tanh, chamber update, output accumulation ----
                        nl = s2.tile([128, 2 * nb], BF16, name="nl", tag="nl")
                        nc.scalar.activation(nl, acc[:, :], AF.Tanh,
                                             scale=cons[m][:, 14:15], bias=cons[m][:, 15:16])
                        if ch_next is not None:
                            chn = ch_next[m]
                        else:
                            chn = s2.tile([128, 2 * nb], BF16, name="chl", tag="chl")
                        nc.vector.tensor_scalar(chn[:, :], nl[:, :], cons[m][:, d:d + 1],
                                                None, op0=OP.mult)
                        if d == 0:
                            nc.vector.tensor_scalar(out_t[m][:, :], chn[:, :], wout[d],
                                                    None, op0=OP.mult)
                        else:
                            wch = s2.tile([128, 2 * nb], BF16, name="wch", tag="wch", bufs=1)
                            nc.vector.tensor_scalar(wch[:, :], chn[:, :], wout[d],
                                                    None, op0=OP.mult)
                            nc.vector.tensor_tensor(out_t[m][:, :], out_t[m][:, :],
                                                    wch[:, :], op=OP.add)
                    if ch_next is not None:
                        ch = ch_next

                for m in range(KT):
                    nc.sync.dma_start(out=ore[m * 128:(m + 1) * 128, c0:c0 + nb],
                                      in_=out_t[m][:, H])
                    nc.sync.dma_start(out=oim[m * 128:(m + 1) * 128, c0:c0 + nb],
                                      in_=out_t[m][:, I])
    nc.finalize()
    return nc


def host_prep(coupling_matrix, harmonic_1, harmonic_2, harmonic_3, harmonic_5,
              mixing_scale, mixing_bias):
    damping = (0.1 / (1.0 + np.exp(np.linspace(0.0, 3.0, D)))).astype(np.float32)
    fd = np.stack([np.exp(-damping.astype(np.float64) * float(dd))
                   for dd in range(DEPTH)]).astype(np.float32)      # [7, D]
    w1_0 = (coupling_matrix + harmonic_1).astype(np.float32)
    w1_r = (w1_0 + np.eye(D, dtype=np.float32)).astype(np.float32)
    wmat = np.ascontiguousarray(np.stack([
        w1_0, w1_r, 0.5 * harmonic_2, S3 * harmonic_3, harmonic_5,
    ]).astype(ml_dtypes.bfloat16))
    consts = np.zeros((D, 16), np.float32)
    consts[:, 0:DEPTH] = fd.T
    consts[:, 14] = mixing_scale.astype(np.float32)
    consts[:, 15] = mixing_bias.astype(np.float32)
    return wmat, consts


_NC_CACHE = {}


def _get_nc(n_chunks, nb):
    key = (n_chunks, nb)
    if key not in _NC_CACHE:
        _NC_CACHE[key] = build_program(n_chunks, nb)
    return _NC_CACHE[key]


def kernel(wave_real, wave_imag, coupling_matrix, harmonic_1, harmonic_2,
           harmonic_3, harmonic_5, mixing_scale, mixing_bias):
    wmat, consts = host_prep(coupling_matrix, harmonic_1, harmonic_2,
                             harmonic_3, harmonic_5, mixing_scale, mixing_bias)
    wreT = np.ascontiguousarray(
        np.asarray(wave_real, np.float32).T.astype(ml_dtypes.bfloat16))  # [D, B]
    wimT = np.ascontiguousarray(
        np.asarray(wave_imag, np.float32).T.astype(ml_dtypes.bfloat16))

    nc = _get_nc(BS // NB, NB)
    in_maps = []
    for c in range(N_CORES):
        sl = slice(c * BS, (c + 1) * BS)
        in_maps.append({
            "wre": np.ascontiguousarray(wreT[:, sl]),
            "wim": np.ascontiguousarray(wimT[:, sl]),
            "wmat": wmat,
            "consts": consts,
        })
    res = bass_utils.run_bass_kernel_spmd(nc, in_maps, core_ids=list(range(N_CORES)))
    out = np.empty((2, B, D), np.float32)
    for c in range(N_CORES):
        sl = slice(c * BS, (c + 1) * BS)
        out[0, sl, :] = res.results[c]["ore"].astype(np.float32).T
        out[1, sl, :] = res.results[c]["oim"].astype(np.float32).T
    return out


# revision 8
# speedup vs baseline: 1.2676x; 1.0460x over previous
"""Trainium2 Bass kernel for nn_ChromaticResonance (v2: all-bf16 pipeline).

Reference computation (per batch row, complex wave w of dim D=512):
  7 depths of: y = w@(C+H1) [+ w for d>0, folded as +I into the matrix]
               + 0.25*|w@H2|^2                       (real only)
               + (1/9)*|w@H3|^2 * (w@H3)
               + 0.04*(w@H5)^5 * |w@H5|^-4.8
       nl = tanh(y*scale + bias)  (componentwise re/im)
       w' = exp(-damping*d) * nl
  out = sum_d w_d * w'_d

v2 strategy (8 cores, data parallel over batch; transposed [D, B] layout):
  - bf16 matmuls (1 cyc/row on PE, same as fp32r, but half the SBUF/LDW
    traffic); psum f32; nb=1024 batch cols per chunk, 4 chunks per core.
  - Single-matrix psum waves ([128, 2048] f32 = 4 banks, 2-slot pingpong);
    the ACT engine is the sole psum drainer (Square/Copy -> bf16 sbuf).
  - ALL pointwise in bf16 on SBUF: DVE tensor_tensor runs 2 elem/cyc,
    tensor_scalar 4 elem/cyc; six 1-unit ops parked on Pool. Scale factors
    (0.25 for H2, 1/9 for H3) are folded into the weights host-side.
  - t5 = 0.04*r2^-2.4 via bf16-bit fastpow: ONE tensor_scalar on the
    uint16 bit pattern (t5bits = -2.4*bits + K16), bitcast back. Max err
    ~11% on a term worth 4% of acc -> ~0.3% output. Total measured rel
    err of this pipeline vs f64 reference: ~0.7% (gate 2e-2).
"""

import numpy as np
import ml_dtypes

import concourse.bass as bass
import concourse.mybir as mybir
import concourse.tile as tile
from concourse import bass_utils
from concourse.bacc import Bacc

F32 = mybir.dt.float32
BF16 = mybir.dt.bfloat16
U16 = mybir.dt.uint16
I16 = mybir.dt.int16
AF = mybir.ActivationFunctionType
OP = mybir.AluOpType

B, D, DEPTH = 32768, 512, 7
N_CORES = 8
BS = B // N_CORES          # batch rows per core
NB = 1024                  # batch columns per chunk
KT = D // 128              # 4 partition tiles of the D dim
# bf16-bit fastpow: bits(0.04*x^-2.4) ~= -2.4*bits(x) + K16
K16 = 54657.5
S3 = float(9.0 ** (-1.0 / 3.0))   # folded into H3 so |h3'|^2*h3' = |h3|^2*h3/9


def _dup2(t, nb=NB):
    """Broadcast a [128, nb] AP to [128, 2, nb] (each column read twice)."""
    ap = t.ap
    return bass.AP(tensor=t.tensor, offset=t.offset, ap=[ap[0], [0, 2], ap[1]])


def _as3(t, nb=NB):
    """View a [128, 2*nb] AP as [128, 2, nb]."""
    return t.rearrange("p (two n) -> p two n", two=2)


def build_program(n_chunks=BS // NB, nb=NB):
    nc = Bacc()
    bcols = n_chunks * nb

    wre = nc.dram_tensor("wre", [D, bcols], BF16, kind="ExternalInput")
    wim = nc.dram_tensor("wim", [D, bcols], BF16, kind="ExternalInput")
    wmat = nc.dram_tensor("wmat", [5, D, D], BF16, kind="ExternalInput")
    ident = nc.dram_tensor("ident", [128, 128], BF16, kind="ExternalInput")
    consts = nc.dram_tensor("consts", [D, 16], F32, kind="ExternalInput")
    ore = nc.dram_tensor("ore", [D, bcols], BF16, kind="ExternalOutput")
    oim = nc.dram_tensor("oim", [D, bcols], BF16, kind="ExternalOutput")

    H = slice(0, nb)       # real half of a packed tile
    I = slice(nb, 2 * nb)  # imag half
    wout = np.exp(-np.linspace(0.0, 2.0, DEPTH))
    wout = [float(x) for x in (wout / wout.sum())]

    with tile.TileContext(nc) as tc:
        with (
            tc.tile_pool(name="wpool", bufs=1) as wpool,
            tc.tile_pool(name="chpool", bufs=1) as chpool,
            tc.tile_pool(name="opool", bufs=1) as opool,
            tc.tile_pool(name="ppool", bufs=1, space="PSUM") as ppool,
            tc.tile_pool(name="s2", bufs=2) as s2,      # [128, 2nb] scratch
            tc.tile_pool(name="s1", bufs=2) as s1,      # [128, nb] scratch
        ):
            # ---- load weights + constants (once) ----
            wt = []
            for mi in range(5):
                w = wpool.tile([128, KT, D], BF16, name=f"wt{mi}", tag=f"wt{mi}")
                for k in range(KT):
                    nc.sync.dma_start(out=w[:, k, :], in_=wmat[mi, k * 128:(k + 1) * 128, :])
                wt.append(w)
            cons = []
            for m in range(KT):
                c = wpool.tile([128, 16], F32, name=f"cons{m}", tag=f"cons{m}")
                nc.sync.dma_start(out=c, in_=consts[m * 128:(m + 1) * 128, :])
                cons.append(c)
            idt = wpool.tile([128, 128], BF16, name="idt", tag="idt")
            nc.sync.dma_start(out=idt, in_=ident[:, :])

            for ci in range(n_chunks):
                c0 = ci * nb
                ch = []
                for k in range(KT):
                    t = chpool.tile([128, 2 * nb], BF16, name=f"cha{k}", tag=f"cha{k}")
                    nc.sync.dma_start(out=t[:, H], in_=wre[k * 128:(k + 1) * 128, c0:c0 + nb])
                    nc.sync.dma_start(out=t[:, I], in_=wim[k * 128:(k + 1) * 128, c0:c0 + nb])
                    ch.append(t)
                out_t = [opool.tile([128, 2 * nb], BF16, name=f"out{m}", tag=f"out{m}")
                         for m in range(KT)]

                for d in range(DEPTH):
                    w1 = wt[0] if d == 0 else wt[1]
                    ch_next = None
                    if d < DEPTH - 1:
                        pong = "b" if d % 2 == 0 else "a"
                        ch_next = [chpool.tile([128, 2 * nb], BF16,
                                               name=f"ch{pong}{m}", tag=f"ch{pong}{m}")
                                   for m in range(KT)]

                    for m in range(KT):
                        msl = slice(m * 128, (m + 1) * 128)

                        def wave(lw, ps, stop=True):
                            for j in range(2 * nb // 512):
                                js = slice(j * 512, (j + 1) * 512)
                                for k in range(KT):
                                    nc.tensor.matmul(ps[:, js], lw[:, k, msl],
                                                     ch[k][:, js],
                                                     start=(k == 0),
                                                     stop=(stop and k == KT - 1))

                        # Wave order h5, h2, h3, w1: the h5 z-chain (longest
                        # pointwise tail) starts 3 waves early so it finishes
                        # under the other waves; every additive term lands in
                        # `pre`, which identity-matmuls inject into the w1
                        # psum group. The per-depth serial tail is then just
                        # ident-mm + tanh + chn.
                        ps_h5 = ppool.tile([128, 2 * nb], F32, name="psA", tag="psA")
                        wave(wt[4], ps_h5)
                        d5s = s2.tile([128, 2 * nb], BF16, name="d5s", tag="d5s")
                        nc.scalar.copy(d5s, ps_h5[:, :])
                        ps_h2 = ppool.tile([128, 2 * nb], F32, name="psB", tag="psB")
                        wave(wt[2], ps_h2)
                        sq2h = s1.tile([128, nb], BF16, name="sq2h", tag="sq2h")
                        sq2i = s1.tile([128, nb], BF16, name="sq2i", tag="sq2i")
                        nc.scalar.activation(sq2h, ps_h2[:, H], AF.Square)
                        nc.scalar.activation(sq2i, ps_h2[:, I], AF.Square)

                        # ---- h5 z-chain (DVE/Pool), overlapped with h2/h3/w1 ----
                        sq5 = s2.tile([128, 2 * nb], BF16, name="sq5", tag="sq5")
                        nc.vector.tensor_tensor(sq5[:, :], d5s[:, :], d5s[:, :], op=OP.mult)
                        r2d = s1.tile([128, nb], BF16, name="r2d", tag="r2d")
                        nc.vector.tensor_tensor(r2d, sq5[:, H], sq5[:, I], op=OP.add)
                        t5w = s1.tile([128, nb], I16, name="t5w", tag="t5w")
                        nc.vector.tensor_scalar(t5w, r2d[:, :].bitcast(U16), -2.4, K16,
                                                op0=OP.mult, op1=OP.add)
                        t5 = t5w[:, :].bitcast(BF16)
                        c2r = s1.tile([128, nb], BF16, name="c2r", tag="c2r")
                        nc.vector.tensor_tensor(c2r, sq5[:, H], sq5[:, I], op=OP.subtract)
                        e5 = s1.tile([128, nb], BF16, name="e5", tag="e5")
                        nc.vector.tensor_tensor(e5, d5s[:, H], d5s[:, I], op=OP.mult)
                        e2 = s1.tile([128, nb], BF16, name="e2", tag="e2", bufs=1)
                        nc.vector.tensor_scalar(e2, e5, 2.0, None, op0=OP.mult)
                        c22 = s1.tile([128, nb], BF16, name="c22", tag="c22", bufs=1)
                        nc.vector.tensor_scalar(c22, c2r, 2.0, None, op0=OP.mult)
                        ee4 = s1.tile([128, nb], BF16, name="ee4", tag="ee4", bufs=1)
                        nc.gpsimd.tensor_tensor(ee4, e2, e2, op=OP.mult)
                        sq2r5 = s1.tile([128, nb], BF16, name="sq2r5", tag="sq2r5", bufs=1)
                        nc.gpsimd.tensor_tensor(sq2r5, c2r, c2r, op=OP.mult)
                        mc4r = s1.tile([128, nb], BF16, name="mc4r", tag="mc4r", bufs=1)
                        nc.vector.tensor_tensor(mc4r, ee4, sq2r5, op=OP.subtract)
                        c4i4 = s1.tile([128, nb], BF16, name="c4i4", tag="c4i4", bufs=1)
                        nc.gpsimd.tensor_tensor(c4i4, e2, c22, op=OP.mult)
                        td5 = s2.tile([128, 2 * nb], BF16, name="td5", tag="td5", bufs=1)
                        nc.vector.tensor_tensor(_as3(td5), _dup2(t5), _as3(d5s), op=OP.mult)
                        # p5hn = -(p5 real); p5i = p5 imag (mc4r = -c4r*... sign flip)
                        q1 = s1.tile([128, nb], BF16, name="q1", tag="q1", bufs=1)
                        q2 = s1.tile([128, nb], BF16, name="q2", tag="q2", bufs=1)
                        q3 = s1.tile([128, nb], BF16, name="q3", tag="q3", bufs=1)
                        q4 = s1.tile([128, nb], BF16, name="q4", tag="q4", bufs=1)
                        nc.vector.tensor_tensor(q1, mc4r, td5[:, H], op=OP.mult)
                        nc.gpsimd.tensor_tensor(q2, c4i4, td5[:, I], op=OP.mult)
                        p5hn = s1.tile([128, nb], BF16, name="p5hn", tag="p5hn", bufs=1)
                        nc.vector.tensor_tensor(p5hn, q1, q2, op=OP.add)
                        nc.gpsimd.tensor_tensor(q3, c4i4, td5[:, H], op=OP.mult)
                        nc.vector.tensor_tensor(q4, mc4r, td5[:, I], op=OP.mult)
                        p5i = s1.tile([128, nb], BF16, name="p5i", tag="p5i", bufs=1)
                        nc.vector.tensor_tensor(p5i, q3, q4, op=OP.subtract)

                        ps_h3 = ppool.tile([128, 2 * nb], F32, name="psA", tag="psA")
                        wave(wt[3], ps_h3)
                        d3s = s2.tile([128, 2 * nb], BF16, name="d3s", tag="d3s")
                        sq3 = s2.tile([128, 2 * nb], BF16, name="sq3", tag="sq3")
                        nc.scalar.copy(d3s, ps_h3[:, :])
                        nc.scalar.activation(sq3, ps_h3[:, :], AF.Square)
                        ps_w1 = ppool.tile([128, 2 * nb], F32, name="psB", tag="psB")
                        wave(w1, ps_w1, stop=False)

                        # ---- pre = all additive harmonic terms, one tile ----
                        r2a = s1.tile([128, nb], BF16, name="r2a", tag="r2a")
                        nc.vector.tensor_tensor(r2a, sq2h, sq2i, op=OP.add)
                        rh = s1.tile([128, nb], BF16, name="rh", tag="rh", bufs=1)
                        nc.vector.tensor_tensor(rh, r2a, p5hn, op=OP.subtract)
                        r2b = s1.tile([128, nb], BF16, name="r2b", tag="r2b")
                        nc.vector.tensor_tensor(r2b, sq3[:, H], sq3[:, I], op=OP.add)
                        ht = s2.tile([128, 2 * nb], BF16, name="ht", tag="ht", bufs=1)
                        nc.vector.tensor_tensor(_as3(ht), _dup2(r2b[:, :]), _as3(d3s),
                                                op=OP.mult)
                        pre = s2.tile([128, 2 * nb], BF16, name="pre", tag="pre")
                        nc.vector.tensor_tensor(pre[:, H], ht[:, H], rh, op=OP.add)
                        nc.vector.tensor_tensor(pre[:, I], ht[:, I], p5i, op=OP.add)
                        for j in range(2 * nb // 512):
                            js = slice(j * 512, (j + 1) * 512)
                            nc.tensor.matmul(ps_w1[:, js], idt[:, :], pre[:, js],
                                             start=False, stop=True)

                        # ---- tanh, chamber update, output accumulation ----
                        nl = s2.tile([128, 2 * nb], BF16, name="nl", tag="nl")
                        nc.scalar.activation(nl, ps_w1[:, :], AF.Tanh,
                                             scale=cons[m][:, 14:15], bias=cons[m][:, 15:16])
                        if ch_next is not None:
                            chn = ch_next[m]
                        else:
                            chn = s2.tile([128, 2 * nb], BF16, name="chl", tag="chl")
                        nc.vector.tensor_scalar(chn[:, :], nl[:, :], cons[m][:, d:d + 1],
                                                None, op0=OP.mult)
                        if d == 0:
                            nc.vector.tensor_scalar(out_t[m][:, :], chn[:, :], wout[d],
                                                    None, op0=OP.mult)
                        else:
                            wch = s2.tile([128, 2 * nb], BF16, name="wch", tag="wch", bufs=1)
                            nc.vector.tensor_scalar(wch[:, :], chn[:, :], wout[d],
                                                    None, op0=OP.mult)
                            nc.vector.tensor_tensor(out_t[m][:, :], out_t[m][:, :],
                                                    wch[:, :], op=OP.add)
                    if ch_next is not None:
                        ch = ch_next

                for m in range(KT):
                    nc.sync.dma_start(out=ore[m * 128:(m + 1) * 128, c0:c0 + nb],
                                      in_=out_t[m][:, H])
                    nc.sync.dma_start(out=oim[m * 128:(m + 1) * 128, c0:c0 + nb],
                                      in_=out_t[m][:, I])
    nc.finalize()
    return nc


def host_prep(coupling_matrix, harmonic_1, harmonic_2, harmonic_3, harmonic_5,
              mixing_scale, mixing_bias):
    damping = (0.1 / (1.0 + np.exp(np.linspace(0.0, 3.0, D)))).astype(np.float32)
    fd = np.stack([np.exp(-damping.astype(np.float64) * float(dd))
                   for dd in range(DEPTH)]).astype(np.float32)      # [7, D]
    w1_0 = (coupling_matrix + harmonic_1).astype(np.float32)
    w1_r = (w1_0 + np.eye(D, dtype=np.float32)).astype(np.float32)
    wmat = np.ascontiguousarray(np.stack([
        w1_0, w1_r, 0.5 * harmonic_2, S3 * harmonic_3, harmonic_5,
    ]).astype(ml_dtypes.bfloat16))
    consts = np.zeros((D, 16), np.float32)
    consts[:, 0:DEPTH] = fd.T
    consts[:, 14] = mixing_scale.astype(np.float32)
    consts[:, 15] = mixing_bias.astype(np.float32)
    return wmat, consts


_IDENT = np.ascontiguousarray(np.eye(128, dtype=np.float32).astype(ml_dtypes.bfloat16))

_NC_CACHE = {}


def _get_nc(n_chunks, nb):
    key = (n_chunks, nb)
    if key not in _NC_CACHE:
        _NC_CACHE[key] = build_program(n_chunks, nb)
    return _NC_CACHE[key]


def kernel(wave_real, wave_imag, coupling_matrix, harmonic_1, harmonic_2,
           harmonic_3, harmonic_5, mixing_scale, mixing_bias):
    wmat, consts = host_prep(coupling_matrix, harmonic_1, harmonic_2,
                             harmonic_3, harmonic_5, mixing_scale, mixing_bias)
    wreT = np.ascontiguousarray(
        np.asarray(wave_real, np.float32).T.astype(ml_dtypes.bfloat16))  # [D, B]
    wimT = np.ascontiguousarray(
        np.asarray(wave_imag, np.float32).T.astype(ml_dtypes.bfloat16))

    nc = _get_nc(BS // NB, NB)
    in_maps = []
    for c in range(N_CORES):
        sl = slice(c * BS, (c + 1) * BS)
        in_maps.append({
            "wre": np.ascontiguousarray(wreT[:, sl]),
            "wim": np.ascontiguousarray(wimT[:, sl]),
            "wmat": wmat,
            "ident": _IDENT,
            "consts": consts,
        })
    res = bass_utils.run_bass_kernel_spmd(nc, in_maps, core_ids=list(range(N_CORES)))
    out = np.empty((2, B, D), np.float32)
    for c in range(N_CORES):
        sl = slice(c * BS, (c + 1) * BS)
        out[0, sl, :] = res.results[c]["ore"].astype(np.float32).T
        out[1, sl, :] = res.results[c]["oim"].astype(np.float32).T
    return out


# revision 9
# speedup vs baseline: 1.6535x; 1.3045x over previous
"""Trainium2 Bass kernel for nn_ChromaticResonance (v2: all-bf16 pipeline).

Reference computation (per batch row, complex wave w of dim D=512):
  7 depths of: y = w@(C+H1) [+ w for d>0, folded as +I into the matrix]
               + 0.25*|w@H2|^2                       (real only)
               + (1/9)*|w@H3|^2 * (w@H3)
               + 0.04*(w@H5)^5 * |w@H5|^-4.8
       nl = tanh(y*scale + bias)  (componentwise re/im)
       w' = exp(-damping*d) * nl
  out = sum_d w_d * w'_d

v2 strategy (8 cores, data parallel over batch; transposed [D, B] layout):
  - bf16 matmuls (1 cyc/row on PE, same as fp32r, but half the SBUF/LDW
    traffic); psum f32; nb=1024 batch cols per chunk, 4 chunks per core.
  - Single-matrix psum waves ([128, 2048] f32 = 4 banks, 2-slot pingpong);
    the ACT engine is the sole psum drainer (Square/Copy -> bf16 sbuf).
  - ALL pointwise in bf16 on SBUF: DVE tensor_tensor runs 2 elem/cyc,
    tensor_scalar 4 elem/cyc; six 1-unit ops parked on Pool. Scale factors
    (0.25 for H2, 1/9 for H3) are folded into the weights host-side.
  - t5 = 0.04*r2^-2.4 via bf16-bit fastpow: ONE tensor_scalar on the
    uint16 bit pattern (t5bits = -2.4*bits + K16), bitcast back. Max err
    ~11% on a term worth 4% of acc -> ~0.3% output. Total measured rel
    err of this pipeline vs f64 reference: ~0.7% (gate 2e-2).
"""

import numpy as np
import ml_dtypes

import concourse.bass as bass
import concourse.mybir as mybir
import concourse.tile as tile
from concourse import bass_utils
from concourse.bacc import Bacc

F32 = mybir.dt.float32
BF16 = mybir.dt.bfloat16
U16 = mybir.dt.uint16
I16 = mybir.dt.int16
AF = mybir.ActivationFunctionType
OP = mybir.AluOpType

B, D, DEPTH = 32768, 512, 7
N_CORES = 8
BS = B // N_CORES          # batch rows per core
NB = 1024                  # batch columns per chunk
KT = D // 128              # 4 partition tiles of the D dim
# bf16-bit fastpow: bits(0.04*x^-2.4) ~= -2.4*bits(x) + K16
K16 = 54657.5
S3 = float(9.0 ** (-1.0 / 3.0))   # folded into H3 so |h3'|^2*h3' = |h3|^2*h3/9


def _dup2(t, nb=NB):
    """Broadcast a [128, nb] AP to [128, 2, nb] (each column read twice)."""
    ap = t.ap
    return bass.AP(tensor=t.tensor, offset=t.offset, ap=[ap[0], [0, 2], ap[1]])


def _as3(t, nb=NB):
    """View a [128, 2*nb] AP as [128, 2, nb]."""
    return t.rearrange("p (two n) -> p two n", two=2)


def build_program(n_chunks=BS // NB, nb=NB):
    nc = Bacc()
    bcols = n_chunks * nb

    wre = nc.dram_tensor("wre", [D, bcols], BF16, kind="ExternalInput")
    wim = nc.dram_tensor("wim", [D, bcols], BF16, kind="ExternalInput")
    wmat = nc.dram_tensor("wmat", [5, D, D], BF16, kind="ExternalInput")
    ident = nc.dram_tensor("ident", [128, 128], BF16, kind="ExternalInput")
    consts = nc.dram_tensor("consts", [D, 16], F32, kind="ExternalInput")
    ore = nc.dram_tensor("ore", [D, bcols], BF16, kind="ExternalOutput")
    oim = nc.dram_tensor("oim", [D, bcols], BF16, kind="ExternalOutput")

    H = slice(0, nb)       # real half of a packed tile
    I = slice(nb, 2 * nb)  # imag half
    wout = np.exp(-np.linspace(0.0, 2.0, DEPTH))
    wout = [float(x) for x in (wout / wout.sum())]

    with tile.TileContext(nc) as tc:
        with (
            tc.tile_pool(name="wpool", bufs=1) as wpool,
            tc.tile_pool(name="chpool", bufs=1) as chpool,
            tc.tile_pool(name="opool", bufs=1) as opool,
            tc.tile_pool(name="ppool", bufs=1, space="PSUM") as ppool,
            tc.tile_pool(name="s2", bufs=2) as s2,      # [128, 2nb] scratch
            tc.tile_pool(name="s1", bufs=2) as s1,      # [128, nb] scratch
        ):
            # ---- load weights + constants (once) ----
            wt = []
            for mi in range(5):
                w = wpool.tile([128, KT, D], BF16, name=f"wt{mi}", tag=f"wt{mi}")
                for k in range(KT):
                    nc.sync.dma_start(out=w[:, k, :], in_=wmat[mi, k * 128:(k + 1) * 128, :])
                wt.append(w)
            cons = []
            for m in range(KT):
                c = wpool.tile([128, 16], F32, name=f"cons{m}", tag=f"cons{m}")
                nc.sync.dma_start(out=c, in_=consts[m * 128:(m + 1) * 128, :])
                cons.append(c)
            idt = wpool.tile([128, 128], BF16, name="idt", tag="idt")
            nc.sync.dma_start(out=idt, in_=ident[:, :])

            for ci in range(n_chunks):
                c0 = ci * nb
                ch = []
                for k in range(KT):
                    t = chpool.tile([128, 2 * nb], BF16, name=f"cha{k}", tag=f"cha{k}")
                    nc.sync.dma_start(out=t[:, H], in_=wre[k * 128:(k + 1) * 128, c0:c0 + nb])
                    nc.sync.dma_start(out=t[:, I], in_=wim[k * 128:(k + 1) * 128, c0:c0 + nb])
                    ch.append(t)
                out_t = [opool.tile([128, 2 * nb], BF16, name=f"out{m}", tag=f"out{m}")
                         for m in range(KT)]

                for d in range(DEPTH):
                    w1 = wt[0] if d == 0 else wt[1]
                    ch_next = None
                    if d < DEPTH - 1:
                        pong = "b" if d % 2 == 0 else "a"
                        ch_next = [chpool.tile([128, 2 * nb], BF16,
                                               name=f"ch{pong}{m}", tag=f"ch{pong}{m}")
                                   for m in range(KT)]

                    for m in range(KT):
                        msl = slice(m * 128, (m + 1) * 128)

                        def wave(lw, ps, stop=True):
                            for j in range(2 * nb // 512):
                                js = slice(j * 512, (j + 1) * 512)
                                for k in range(KT):
                                    nc.tensor.matmul(ps[:, js], lw[:, k, msl],
                                                     ch[k][:, js],
                                                     start=(k == 0),
                                                     stop=(stop and k == KT - 1))

                        # Wave order h5, h2, h3, w1: the h5 z-chain (longest
                        # pointwise tail) starts 3 waves early so it finishes
                        # under the other waves; every additive term lands in
                        # `pre`, which identity-matmuls inject into the w1
                        # psum group. The per-depth serial tail is then just
                        # ident-mm + tanh + chn.
                        # NOTE: the Pool engine is banned from this pipeline -
                        # measured on hw, any concurrent Pool op demotes DVE
                        # throughput to 0.25-0.5x (SBUF port starvation).
                        ps_h5 = ppool.tile([128, 2 * nb], F32, name="psA", tag="psA")
                        wave(wt[4], ps_h5)
                        d5s = s2.tile([128, 2 * nb], BF16, name="d5s", tag="d5s")
                        nc.scalar.copy(d5s, ps_h5[:, :])
                        sq5 = s2.tile([128, 2 * nb], BF16, name="sq5", tag="sq5")
                        nc.scalar.activation(sq5, ps_h5[:, :], AF.Square)
                        ps_h2 = ppool.tile([128, 2 * nb], F32, name="psB", tag="psB")
                        wave(wt[2], ps_h2)
                        sq2h = s1.tile([128, nb], BF16, name="sq2h", tag="sq2h")
                        sq2i = s1.tile([128, nb], BF16, name="sq2i", tag="sq2i")
                        nc.scalar.activation(sq2h, ps_h2[:, H], AF.Square)
                        nc.scalar.activation(sq2i, ps_h2[:, I], AF.Square)

                        # ---- h5 z-chain (DVE only), overlapped with h2/h3/w1 ----
                        r2d = s1.tile([128, nb], BF16, name="r2d", tag="r2d")
                        nc.vector.tensor_tensor(r2d, sq5[:, H], sq5[:, I], op=OP.add)
                        t5w = s1.tile([128, nb], I16, name="t5w", tag="t5w")
                        nc.vector.tensor_scalar(t5w, r2d[:, :].bitcast(U16), -2.4, K16,
                                                op0=OP.mult, op1=OP.add)
                        t5 = t5w[:, :].bitcast(BF16)
                        c2r = s1.tile([128, nb], BF16, name="c2r", tag="c2r")
                        nc.vector.tensor_tensor(c2r, sq5[:, H], sq5[:, I], op=OP.subtract)
                        e5 = s1.tile([128, nb], BF16, name="e5", tag="e5")
                        nc.vector.tensor_tensor(e5, d5s[:, H], d5s[:, I], op=OP.mult)
                        ee4 = s1.tile([128, nb], BF16, name="ee4", tag="ee4", bufs=1)
                        nc.vector.scalar_tensor_tensor(ee4, e5, 4.0, e5,
                                                       op0=OP.mult, op1=OP.mult)
                        sq2r5 = s1.tile([128, nb], BF16, name="sq2r5", tag="sq2r5", bufs=1)
                        nc.vector.tensor_tensor(sq2r5, c2r, c2r, op=OP.mult)
                        mc4r = s1.tile([128, nb], BF16, name="mc4r", tag="mc4r", bufs=1)
                        nc.vector.tensor_tensor(mc4r, ee4, sq2r5, op=OP.subtract)
                        c4i4 = s1.tile([128, nb], BF16, name="c4i4", tag="c4i4", bufs=1)
                        nc.vector.scalar_tensor_tensor(c4i4, e5, 4.0, c2r,
                                                       op0=OP.mult, op1=OP.mult)
                        td5 = s2.tile([128, 2 * nb], BF16, name="td5", tag="td5", bufs=1)
                        nc.vector.tensor_tensor(_as3(td5), _dup2(t5), _as3(d5s), op=OP.mult)
                        # p5hn = -(p5 real); p5i = p5 imag (mc4r = -c4r*... sign flip)
                        q1 = s1.tile([128, nb], BF16, name="q1", tag="q1", bufs=1)
                        q2 = s1.tile([128, nb], BF16, name="q2", tag="q2", bufs=1)
                        q3 = s1.tile([128, nb], BF16, name="q3", tag="q3", bufs=1)
                        q4 = s1.tile([128, nb], BF16, name="q4", tag="q4", bufs=1)
                        nc.vector.tensor_tensor(q1, mc4r, td5[:, H], op=OP.mult)
                        nc.vector.tensor_tensor(q2, c4i4, td5[:, I], op=OP.mult)
                        p5hn = s1.tile([128, nb], BF16, name="p5hn", tag="p5hn", bufs=1)
                        nc.vector.tensor_tensor(p5hn, q1, q2, op=OP.add)
                        nc.vector.tensor_tensor(q3, c4i4, td5[:, H], op=OP.mult)
                        nc.vector.tensor_tensor(q4, mc4r, td5[:, I], op=OP.mult)
                        p5i = s1.tile([128, nb], BF16, name="p5i", tag="p5i", bufs=1)
                        nc.vector.tensor_tensor(p5i, q3, q4, op=OP.subtract)

                        ps_h3 = ppool.tile([128, 2 * nb], F32, name="psA", tag="psA")
                        wave(wt[3], ps_h3)
                        d3s = s2.tile([128, 2 * nb], BF16, name="d3s", tag="d3s")
                        sq3 = s2.tile([128, 2 * nb], BF16, name="sq3", tag="sq3")
                        nc.scalar.copy(d3s, ps_h3[:, :])
                        nc.scalar.activation(sq3, ps_h3[:, :], AF.Square)
                        ps_w1 = ppool.tile([128, 2 * nb], F32, name="psB", tag="psB")
                        wave(w1, ps_w1, stop=False)

                        # ---- pre = all additive harmonic terms, one tile ----
                        r2a = s1.tile([128, nb], BF16, name="r2a", tag="r2a")
                        nc.vector.tensor_tensor(r2a, sq2h, sq2i, op=OP.add)
                        rh = s1.tile([128, nb], BF16, name="rh", tag="rh", bufs=1)
                        nc.vector.tensor_tensor(rh, r2a, p5hn, op=OP.subtract)
                        r2b = s1.tile([128, nb], BF16, name="r2b", tag="r2b")
                        nc.vector.tensor_tensor(r2b, sq3[:, H], sq3[:, I], op=OP.add)
                        pre = s2.tile([128, 2 * nb], BF16, name="pre", tag="pre")
                        nc.vector.tensor_tensor(_as3(pre), _dup2(r2b[:, :]), _as3(d3s),
                                                op=OP.mult)
                        nc.vector.tensor_tensor(pre[:, H], pre[:, H], rh, op=OP.add)
                        nc.vector.tensor_tensor(pre[:, I], pre[:, I], p5i, op=OP.add)
                        for j in range(2 * nb // 512):
                            js = slice(j * 512, (j + 1) * 512)
                            nc.tensor.matmul(ps_w1[:, js], idt[:, :], pre[:, js],
                                             start=False, stop=True)

                        # ---- tanh, chamber update, output accumulation ----
                        nl = s2.tile([128, 2 * nb], BF16, name="nl", tag="nl")
                        nc.scalar.activation(nl, ps_w1[:, :], AF.Tanh,
                                             scale=cons[m][:, 14:15], bias=cons[m][:, 15:16])
                        if ch_next is not None:
                            chn = ch_next[m]
                        else:
                            chn = s2.tile([128, 2 * nb], BF16, name="chl", tag="chl")
                        nc.scalar.mul(chn[:, :], nl[:, :], cons[m][:, d:d + 1])
                        if d == 0:
                            nc.vector.tensor_scalar(out_t[m][:, :], chn[:, :], wout[d],
                                                    None, op0=OP.mult)
                        else:
                            wch = s2.tile([128, 2 * nb], BF16, name="wch", tag="wch", bufs=1)
                            nc.vector.tensor_scalar(wch[:, :], chn[:, :], wout[d],
                                                    None, op0=OP.mult)
                            nc.vector.tensor_tensor(out_t[m][:, :], out_t[m][:, :],
                                                    wch[:, :], op=OP.add)
                    if ch_next is not None:
                        ch = ch_next

                for m in range(KT):
                    nc.sync.dma_start(out=ore[m * 128:(m + 1) * 128, c0:c0 + nb],
                                      in_=out_t[m][:, H])
                    nc.sync.dma_start(out=oim[m * 128:(m + 1) * 128, c0:c0 + nb],
                                      in_=out_t[m][:, I])
    nc.finalize()
    return nc


def host_prep(coupling_matrix, harmonic_1, harmonic_2, harmonic_3, harmonic_5,
              mixing_scale, mixing_bias):
    damping = (0.1 / (1.0 + np.exp(np.linspace(0.0, 3.0, D)))).astype(np.float32)
    fd = np.stack([np.exp(-damping.astype(np.float64) * float(dd))
                   for dd in range(DEPTH)]).astype(np.float32)      # [7, D]
    w1_0 = (coupling_matrix + harmonic_1).astype(np.float32)
    w1_r = (w1_0 + np.eye(D, dtype=np.float32)).astype(np.float32)
    wmat = np.ascontiguousarray(np.stack([
        w1_0, w1_r, 0.5 * harmonic_2, S3 * harmonic_3, harmonic_5,
    ]).astype(ml_dtypes.bfloat16))
    consts = np.zeros((D, 16), np.float32)
    consts[:, 0:DEPTH] = fd.T
    consts[:, 14] = mixing_scale.astype(np.float32)
    consts[:, 15] = mixing_bias.astype(np.float32)
    return wmat, consts


_IDENT = np.ascontiguousarray(np.eye(128, dtype=np.float32).astype(ml_dtypes.bfloat16))

_NC_CACHE = {}


def _get_nc(n_chunks, nb):
    key = (n_chunks, nb)
    if key not in _NC_CACHE:
        _NC_CACHE[key] = build_program(n_chunks, nb)
    return _NC_CACHE[key]


def kernel(wave_real, wave_imag, coupling_matrix, harmonic_1, harmonic_2,
           harmonic_3, harmonic_5, mixing_scale, mixing_bias):
    wmat, consts = host_prep(coupling_matrix, harmonic_1, harmonic_2,
                             harmonic_3, harmonic_5, mixing_scale, mixing_bias)
    wreT = np.ascontiguousarray(
        np.asarray(wave_real, np.float32).T.astype(ml_dtypes.bfloat16))  # [D, B]
    wimT = np.ascontiguousarray(
        np.asarray(wave_imag, np.float32).T.astype(ml_dtypes.bfloat16))

    nc = _get_nc(BS // NB, NB)
    in_maps = []
    for c in range(N_CORES):
        sl = slice(c * BS, (c + 1) * BS)
        in_maps.append({
            "wre": np.ascontiguousarray(wreT[:, sl]),
            "wim": np.ascontiguousarray(wimT[:, sl]),
            "wmat": wmat,
            "ident": _IDENT,
            "consts": consts,
        })
    res = bass_utils.run_bass_kernel_spmd(nc, in_maps, core_ids=list(range(N_CORES)))
    out = np.empty((2, B, D), np.float32)
    for c in range(N_CORES):
        sl = slice(c * BS, (c + 1) * BS)
        out[0, sl, :] = res.results[c]["ore"].astype(np.float32).T
        out[1, sl, :] = res.results[c]["oim"].astype(np.float32).T
    return out


# revision 13
# speedup vs baseline: 1.7087x; 1.0334x over previous
"""Trainium2 Bass kernel for nn_ChromaticResonance (v2: all-bf16 pipeline).

Reference computation (per batch row, complex wave w of dim D=512):
  7 depths of: y = w@(C+H1) [+ w for d>0, folded as +I into the matrix]
               + 0.25*|w@H2|^2                       (real only)
               + (1/9)*|w@H3|^2 * (w@H3)
               + 0.04*(w@H5)^5 * |w@H5|^-4.8
       nl = tanh(y*scale + bias)  (componentwise re/im)
       w' = exp(-damping*d) * nl
  out = sum_d w_d * w'_d

v2 strategy (8 cores, data parallel over batch; transposed [D, B] layout):
  - bf16 matmuls (1 cyc/row on PE, same as fp32r, but half the SBUF/LDW
    traffic); psum f32; nb=1024 batch cols per chunk, 4 chunks per core.
  - Single-matrix psum waves ([128, 2048] f32 = 4 banks, 2-slot pingpong);
    the ACT engine is the sole psum drainer (Square/Copy -> bf16 sbuf).
  - ALL pointwise in bf16 on SBUF: DVE tensor_tensor runs 2 elem/cyc,
    tensor_scalar 4 elem/cyc; six 1-unit ops parked on Pool. Scale factors
    (0.25 for H2, 1/9 for H3) are folded into the weights host-side.
  - t5 = 0.04*r2^-2.4 via bf16-bit fastpow: ONE tensor_scalar on the
    uint16 bit pattern (t5bits = -2.4*bits + K16), bitcast back. Max err
    ~11% on a term worth 4% of acc -> ~0.3% output. Total measured rel
    err of this pipeline vs f64 reference: ~0.7% (gate 2e-2).
"""

import numpy as np
import ml_dtypes

import concourse.bass as bass
import concourse.mybir as mybir
import concourse.tile as tile
from concourse import bass_utils
from concourse.bacc import Bacc

F32 = mybir.dt.float32
BF16 = mybir.dt.bfloat16
U16 = mybir.dt.uint16
I16 = mybir.dt.int16
AF = mybir.ActivationFunctionType
OP = mybir.AluOpType

B, D, DEPTH = 32768, 512, 7
N_CORES = 8
BS = B // N_CORES          # batch rows per core
NB = 1024                  # batch columns per chunk
KT = D // 128              # 4 partition tiles of the D dim
# bf16-bit fastpow: bits(0.04*x^-2.4) ~= -2.4*bits(x) + K16
K16 = 54657.5
S3 = float(9.0 ** (-1.0 / 3.0))   # folded into H3 so |h3'|^2*h3' = |h3|^2*h3/9


def _dup2(t, nb=NB):
    """Broadcast a [128, nb] AP to [128, 2, nb] (each column read twice)."""
    ap = t.ap
    return bass.AP(tensor=t.tensor, offset=t.offset, ap=[ap[0], [0, 2], ap[1]])


def _as3(t, nb=NB):
    """View a [128, 2*nb] AP as [128, 2, nb]."""
    return t.rearrange("p (two n) -> p two n", two=2)


def build_program(n_chunks=BS // NB, nb=NB):
    nc = Bacc()
    bcols = n_chunks * nb

    wre = nc.dram_tensor("wre", [D, bcols], BF16, kind="ExternalInput")
    wim = nc.dram_tensor("wim", [D, bcols], BF16, kind="ExternalInput")
    wmat = nc.dram_tensor("wmat", [5, D, D], BF16, kind="ExternalInput")
    ident = nc.dram_tensor("ident", [2, 128, 128], BF16, kind="ExternalInput")
    consts = nc.dram_tensor("consts", [D, 16], F32, kind="ExternalInput")
    ore = nc.dram_tensor("ore", [D, bcols], BF16, kind="ExternalOutput")
    oim = nc.dram_tensor("oim", [D, bcols], BF16, kind="ExternalOutput")

    H = slice(0, nb)       # real half of a packed tile
    I = slice(nb, 2 * nb)  # imag half
    wout = np.exp(-np.linspace(0.0, 2.0, DEPTH))
    wout = [float(x) for x in (wout / wout.sum())]

    with tile.TileContext(nc) as tc:
        with (
            tc.tile_pool(name="wpool", bufs=1) as wpool,
            tc.tile_pool(name="chpool", bufs=1) as chpool,
            tc.tile_pool(name="opool", bufs=1) as opool,
            tc.tile_pool(name="ppool", bufs=1, space="PSUM") as ppool,
            tc.tile_pool(name="s2", bufs=2) as s2,      # [128, 2nb] scratch
            tc.tile_pool(name="s1", bufs=2) as s1,      # [128, nb] scratch
        ):
            # ---- load weights + constants (once) ----
            wt = []
            for mi in range(5):
                w = wpool.tile([128, KT, D], BF16, name=f"wt{mi}", tag=f"wt{mi}")
                for k in range(KT):
                    nc.sync.dma_start(out=w[:, k, :], in_=wmat[mi, k * 128:(k + 1) * 128, :])
                wt.append(w)
            cons = []
            for m in range(KT):
                c = wpool.tile([128, 16], F32, name=f"cons{m}", tag=f"cons{m}")
                nc.sync.dma_start(out=c, in_=consts[m * 128:(m + 1) * 128, :])
                cons.append(c)
            idt = wpool.tile([128, 128], BF16, name="idt", tag="idt")
            nc.sync.dma_start(out=idt, in_=ident[0, :, :])
            nidt = wpool.tile([128, 128], BF16, name="nidt", tag="nidt")
            nc.sync.dma_start(out=nidt, in_=ident[1, :, :])

            for ci in range(n_chunks):
                c0 = ci * nb
                ch = []
                for k in range(KT):
                    t = chpool.tile([128, 2 * nb], BF16, name=f"cha{k}", tag=f"cha{k}")
                    nc.sync.dma_start(out=t[:, H], in_=wre[k * 128:(k + 1) * 128, c0:c0 + nb])
                    nc.sync.dma_start(out=t[:, I], in_=wim[k * 128:(k + 1) * 128, c0:c0 + nb])
                    ch.append(t)
                out_t = [opool.tile([128, 2 * nb], BF16, name=f"out{m}", tag=f"out{m}")
                         for m in range(KT)]

                for d in range(DEPTH):
                    w1 = wt[0] if d == 0 else wt[1]
                    ch_next = None
                    if d < DEPTH - 1:
                        pong = "b" if d % 2 == 0 else "a"
                        ch_next = [chpool.tile([128, 2 * nb], BF16,
                                               name=f"ch{pong}{m}", tag=f"ch{pong}{m}")
                                   for m in range(KT)]

                    for m in range(KT):
                        msl = slice(m * 128, (m + 1) * 128)

                        def wave(lw, ps, stop=True):
                            for j in range(2 * nb // 512):
                                js = slice(j * 512, (j + 1) * 512)
                                for k in range(KT):
                                    nc.tensor.matmul(ps[:, js], lw[:, k, msl],
                                                     ch[k][:, js],
                                                     start=(k == 0),
                                                     stop=(stop and k == KT - 1))

                        # Wave order h5, h2, h3, w1: the h5 z-chain (longest
                        # pointwise tail) starts 3 waves early so it finishes
                        # under the other waves; every additive term lands in
                        # `pre`, which identity-matmuls inject into the w1
                        # psum group. The per-depth serial tail is then just
                        # ident-mm + tanh + chn.
                        # NOTE: the Pool engine is banned from this pipeline -
                        # measured on hw, any concurrent Pool op demotes DVE
                        # throughput to 0.25-0.5x (SBUF port starvation).
                        ps_h5 = ppool.tile([128, 2 * nb], F32, name="psA", tag="psA")
                        wave(wt[4], ps_h5)
                        d5s = s2.tile([128, 2 * nb], BF16, name="d5s", tag="d5s")
                        nc.scalar.copy(d5s, ps_h5[:, :])

                        # ---- h5 z-chain (DVE only), overlapped with h3/h2/w1 ----
                        sq5 = s2.tile([128, 2 * nb], BF16, name="sq5", tag="sq5")
                        nc.vector.tensor_tensor(sq5[:, :], d5s[:, :], d5s[:, :], op=OP.mult)
                        r2d = s1.tile([128, nb], BF16, name="r2d", tag="r2d")
                        nc.vector.tensor_tensor(r2d, sq5[:, H], sq5[:, I], op=OP.add)
                        t5w = s1.tile([128, nb], I16, name="t5w", tag="t5w")
                        nc.vector.tensor_scalar(t5w, r2d[:, :].bitcast(U16), -2.4, K16,
                                                op0=OP.mult, op1=OP.add)
                        t5 = t5w[:, :].bitcast(BF16)
                        c2r = s1.tile([128, nb], BF16, name="c2r", tag="c2r")
                        nc.vector.tensor_tensor(c2r, sq5[:, H], sq5[:, I], op=OP.subtract)
                        e5 = s1.tile([128, nb], BF16, name="e5", tag="e5")
                        nc.vector.tensor_tensor(e5, d5s[:, H], d5s[:, I], op=OP.mult)
                        e2 = s1.tile([128, nb], BF16, name="e2", tag="e2", bufs=1)
                        nc.vector.tensor_scalar(e2, e5, 2.0, None, op0=OP.mult)
                        c22 = s1.tile([128, nb], BF16, name="c22", tag="c22", bufs=1)
                        nc.vector.tensor_scalar(c22, c2r, 2.0, None, op0=OP.mult)
                        ee4 = s1.tile([128, nb], BF16, name="ee4", tag="ee4", bufs=1)
                        nc.vector.tensor_tensor(ee4, e2, e2, op=OP.mult)
                        c4i4 = s1.tile([128, nb], BF16, name="c4i4", tag="c4i4", bufs=1)
                        nc.vector.tensor_tensor(c4i4, e2, c22, op=OP.mult)
                        sq2r5 = s1.tile([128, nb], BF16, name="sq2r5", tag="sq2r5", bufs=1)
                        nc.scalar.activation(sq2r5, c2r, AF.Square)
                        mc4r = s1.tile([128, nb], BF16, name="mc4r", tag="mc4r", bufs=1)
                        nc.vector.tensor_tensor(mc4r, ee4, sq2r5, op=OP.subtract)
                        td5 = s2.tile([128, 2 * nb], BF16, name="td5", tag="td5", bufs=1)
                        nc.vector.tensor_tensor(_as3(td5), _dup2(t5), _as3(d5s), op=OP.mult)
                        # p5hn = -(p5 real); p5i = p5 imag (mc4r = -c4r*... sign flip)
                        q1 = s1.tile([128, nb], BF16, name="q1", tag="q1", bufs=1)
                        q2 = s1.tile([128, nb], BF16, name="q2", tag="q2", bufs=1)
                        q3 = s1.tile([128, nb], BF16, name="q3", tag="q3", bufs=1)
                        q4 = s1.tile([128, nb], BF16, name="q4", tag="q4", bufs=1)
                        nc.vector.tensor_tensor(q1, mc4r, td5[:, H], op=OP.mult)
                        nc.vector.tensor_tensor(q2, c4i4, td5[:, I], op=OP.mult)
                        p5hn = s1.tile([128, nb], BF16, name="p5hn", tag="p5hn", bufs=1)
                        nc.vector.tensor_tensor(p5hn, q1, q2, op=OP.add)
                        nc.vector.tensor_tensor(q3, c4i4, td5[:, H], op=OP.mult)
                        nc.vector.tensor_tensor(q4, mc4r, td5[:, I], op=OP.mult)
                        p5i = s1.tile([128, nb], BF16, name="p5i", tag="p5i", bufs=1)
                        nc.vector.tensor_tensor(p5i, q3, q4, op=OP.subtract)

                        ps_h3 = ppool.tile([128, 2 * nb], F32, name="psB", tag="psB")
                        wave(wt[3], ps_h3)
                        d3s = s2.tile([128, 2 * nb], BF16, name="d3s", tag="d3s")
                        sq3 = s2.tile([128, 2 * nb], BF16, name="sq3", tag="sq3")
                        nc.scalar.copy(d3s, ps_h3[:, :])
                        nc.scalar.activation(sq3, ps_h3[:, :], AF.Square)
                        r2b = s1.tile([128, nb], BF16, name="r2b", tag="r2b")
                        nc.vector.tensor_tensor(r2b, sq3[:, H], sq3[:, I], op=OP.add)
                        ht = s2.tile([128, 2 * nb], BF16, name="ht", tag="ht", bufs=1)
                        nc.vector.tensor_tensor(_as3(ht), _dup2(r2b[:, :]), _as3(d3s),
                                                op=OP.mult)

                        ps_h2 = ppool.tile([128, 2 * nb], F32, name="psA", tag="psA")
                        wave(wt[2], ps_h2)
                        sq2h = s1.tile([128, nb], BF16, name="sq2h", tag="sq2h")
                        sq2i = s1.tile([128, nb], BF16, name="sq2i", tag="sq2i")
                        nc.scalar.activation(sq2h, ps_h2[:, H], AF.Square)
                        nc.scalar.activation(sq2i, ps_h2[:, I], AF.Square)
                        r2a = s1.tile([128, nb], BF16, name="r2a", tag="r2a")
                        nc.vector.tensor_tensor(r2a, sq2h, sq2i, op=OP.add)

                        ps_w1 = ppool.tile([128, 2 * nb], F32, name="psB", tag="psB")
                        wave(w1, ps_w1, stop=False)
                        # Inject the additive terms into the w1 psum group via
                        # +/-identity matmuls: ht (full), +r2a and -p5hn into the
                        # real half, +p5i into the imag half. The last matmul
                        # touching each psum bank carries stop=True.
                        nj = 2 * nb // 512
                        for j in range(nj):
                            js = slice(j * 512, (j + 1) * 512)
                            nc.tensor.matmul(ps_w1[:, js], idt[:, :], ht[:, js],
                                             start=False, stop=False)
                        for j in range(nb // 512):
                            js = slice(j * 512, (j + 1) * 512)
                            nc.tensor.matmul(ps_w1[:, js], idt[:, :], r2a[:, js],
                                             start=False, stop=False)
                        for j in range(nb // 512):
                            js = slice(nb + j * 512, nb + (j + 1) * 512)
                            jr = slice(j * 512, (j + 1) * 512)
                            nc.tensor.matmul(ps_w1[:, js], idt[:, :], p5i[:, jr],
                                             start=False, stop=True)
                        for j in range(nb // 512):
                            js = slice(j * 512, (j + 1) * 512)
                            nc.tensor.matmul(ps_w1[:, js], nidt[:, :], p5hn[:, js],
                                             start=False, stop=True)

                        # ---- tanh, chamber update, output accumulation ----
                        nl = s2.tile([128, 2 * nb], BF16, name="nl", tag="nl")
                        nc.scalar.activation(nl, ps_w1[:, :], AF.Tanh,
                                             scale=cons[m][:, 14:15], bias=cons[m][:, 15:16])
                        if ch_next is not None:
                            chn = ch_next[m]
                        else:
                            chn = s2.tile([128, 2 * nb], BF16, name="chl", tag="chl")
                        nc.vector.tensor_scalar(chn[:, :], nl[:, :], cons[m][:, d:d + 1],
                                                None, op0=OP.mult)
                        if d == 0:
                            nc.vector.tensor_scalar(out_t[m][:, :], chn[:, :], wout[d],
                                                    None, op0=OP.mult)
                        else:
                            wch = s2.tile([128, 2 * nb], BF16, name="wch", tag="wch", bufs=1)
                            nc.vector.tensor_scalar(wch[:, :], chn[:, :], wout[d],
                                                    None, op0=OP.mult)
                            nc.vector.tensor_tensor(out_t[m][:, :], out_t[m][:, :],
                                                    wch[:, :], op=OP.add)
                    if ch_next is not None:
                        ch = ch_next

                for m in range(KT):
                    nc.sync.dma_start(out=ore[m * 128:(m + 1) * 128, c0:c0 + nb],
                                      in_=out_t[m][:, H])
                    nc.sync.dma_start(out=oim[m * 128:(m + 1) * 128, c0:c0 + nb],
                                      in_=out_t[m][:, I])
    nc.finalize()
    return nc


def host_prep(coupling_matrix, harmonic_1, harmonic_2, harmonic_3, harmonic_5,
              mixing_scale, mixing_bias):
    damping = (0.1 / (1.0 + np.exp(np.linspace(0.0, 3.0, D)))).astype(np.float32)
    fd = np.stack([np.exp(-damping.astype(np.float64) * float(dd))
                   for dd in range(DEPTH)]).astype(np.float32)      # [7, D]
    w1_0 = (coupling_matrix + harmonic_1).astype(np.float32)
    w1_r = (w1_0 + np.eye(D, dtype=np.float32)).astype(np.float32)
    wmat = np.ascontiguousarray(np.stack([
        w1_0, w1_r, 0.5 * harmonic_2, S3 * harmonic_3, harmonic_5,
    ]).astype(ml_dtypes.bfloat16))
    consts = np.zeros((D, 16), np.float32)
    consts[:, 0:DEPTH] = fd.T
    consts[:, 14] = mixing_scale.astype(np.float32)
    consts[:, 15] = mixing_bias.astype(np.float32)
    return wmat, consts


_IDENT = np.ascontiguousarray(np.stack([np.eye(128, dtype=np.float32),
                                        -np.eye(128, dtype=np.float32)]).astype(ml_dtypes.bfloat16))

_NC_CACHE = {}


def _get_nc(n_chunks, nb):
    key = (n_chunks, nb)
    if key not in _NC_CACHE:
        _NC_CACHE[key] = build_program(n_chunks, nb)
    return _NC_CACHE[key]


def kernel(wave_real, wave_imag, coupling_matrix, harmonic_1, harmonic_2,
           harmonic_3, harmonic_5, mixing_scale, mixing_bias):
    wmat, consts = host_prep(coupling_matrix, harmonic_1, harmonic_2,
                             harmonic_3, harmonic_5, mixing_scale, mixing_bias)
    wreT = np.ascontiguousarray(
        np.asarray(wave_real, np.float32).T.astype(ml_dtypes.bfloat16))  # [D, B]
    wimT = np.ascontiguousarray(
        np.asarray(wave_imag, np.float32).T.astype(ml_dtypes.bfloat16))

    nc = _get_nc(BS // NB, NB)
    in_maps = []
    for c in range(N_CORES):
        sl = slice(c * BS, (c + 1) * BS)
        in_maps.append({
            "wre": np.ascontiguousarray(wreT[:, sl]),
            "wim": np.ascontiguousarray(wimT[:, sl]),
            "wmat": wmat,
            "ident": _IDENT,
            "consts": consts,
        })
    res = bass_utils.run_bass_kernel_spmd(nc, in_maps, core_ids=list(range(N_CORES)))
    out = np.empty((2, B, D), np.float32)
    for c in range(N_CORES):
        sl = slice(c * BS, (c + 1) * BS)
        out[0, sl, :] = res.results[c]["ore"].astype(np.float32).T
        out[1, sl, :] = res.results[c]["oim"].astype(np.float32).T
    return out


# revision 16
# speedup vs baseline: 1.7584x; 1.0291x over previous
"""Trainium2 Bass kernel for nn_ChromaticResonance (v2: all-bf16 pipeline).

Reference computation (per batch row, complex wave w of dim D=512):
  7 depths of: y = w@(C+H1) [+ w for d>0, folded as +I into the matrix]
               + 0.25*|w@H2|^2                       (real only)
               + (1/9)*|w@H3|^2 * (w@H3)
               + 0.04*(w@H5)^5 * |w@H5|^-4.8
       nl = tanh(y*scale + bias)  (componentwise re/im)
       w' = exp(-damping*d) * nl
  out = sum_d w_d * w'_d

v2 strategy (8 cores, data parallel over batch; transposed [D, B] layout):
  - bf16 matmuls (1 cyc/row on PE, same as fp32r, but half the SBUF/LDW
    traffic); psum f32; nb=1024 batch cols per chunk, 4 chunks per core.
  - Single-matrix psum waves ([128, 2048] f32 = 4 banks, 2-slot pingpong);
    the ACT engine is the sole psum drainer (Square/Copy -> bf16 sbuf).
  - ALL pointwise in bf16 on SBUF: DVE tensor_tensor runs 2 elem/cyc,
    tensor_scalar 4 elem/cyc; six 1-unit ops parked on Pool. Scale factors
    (0.25 for H2, 1/9 for H3) are folded into the weights host-side.
  - t5 = 0.04*r2^-2.4 via bf16-bit fastpow: ONE tensor_scalar on the
    uint16 bit pattern (t5bits = -2.4*bits + K16), bitcast back. Max err
    ~11% on a term worth 4% of acc -> ~0.3% output. Total measured rel
    err of this pipeline vs f64 reference: ~0.7% (gate 2e-2).
"""

import numpy as np
import ml_dtypes

import concourse.bass as bass
import concourse.mybir as mybir
import concourse.tile as tile
from concourse import bass_utils
from concourse.bacc import Bacc

F32 = mybir.dt.float32
BF16 = mybir.dt.bfloat16
U16 = mybir.dt.uint16
I16 = mybir.dt.int16
AF = mybir.ActivationFunctionType
OP = mybir.AluOpType

B, D, DEPTH = 32768, 512, 7
N_CORES = 8
BS = B // N_CORES          # batch rows per core
NB = 1024                  # batch columns per chunk
KT = D // 128              # 4 partition tiles of the D dim
# bf16-bit fastpow: bits(0.04*x^-2.4) ~= -2.4*bits(x) + K16
K16 = 54657.5
S3 = float(9.0 ** (-1.0 / 3.0))   # folded into H3 so |h3'|^2*h3' = |h3|^2*h3/9


def _dup2(t, nb=NB):
    """Broadcast a [128, nb] AP to [128, 2, nb] (each column read twice)."""
    ap = t.ap
    return bass.AP(tensor=t.tensor, offset=t.offset, ap=[ap[0], [0, 2], ap[1]])


def _as3(t, nb=NB):
    """View a [128, 2*nb] AP as [128, 2, nb]."""
    return t.rearrange("p (two n) -> p two n", two=2)


def build_program(n_chunks=BS // NB, nb=NB):
    nc = Bacc()
    bcols = n_chunks * nb

    wre = nc.dram_tensor("wre", [D, bcols], BF16, kind="ExternalInput")
    wim = nc.dram_tensor("wim", [D, bcols], BF16, kind="ExternalInput")
    wmat = nc.dram_tensor("wmat", [5, D, D], BF16, kind="ExternalInput")
    ident = nc.dram_tensor("ident", [2, 128, 128], BF16, kind="ExternalInput")
    consts = nc.dram_tensor("consts", [D, 16], F32, kind="ExternalInput")
    ore = nc.dram_tensor("ore", [D, bcols], BF16, kind="ExternalOutput")
    oim = nc.dram_tensor("oim", [D, bcols], BF16, kind="ExternalOutput")

    H = slice(0, nb)       # real half of a packed tile
    I = slice(nb, 2 * nb)  # imag half
    wout = np.exp(-np.linspace(0.0, 2.0, DEPTH))
    wout = [float(x) for x in (wout / wout.sum())]

    with tile.TileContext(nc) as tc:
        with (
            tc.tile_pool(name="wpool", bufs=1) as wpool,
            tc.tile_pool(name="chpool", bufs=1) as chpool,
            tc.tile_pool(name="opool", bufs=1) as opool,
            tc.tile_pool(name="ppool", bufs=1, space="PSUM") as ppool,
            tc.tile_pool(name="s2", bufs=2) as s2,      # [128, 2nb] scratch
            tc.tile_pool(name="s1", bufs=2) as s1,      # [128, nb] scratch
        ):
            # ---- load weights + constants (once) ----
            wt = []
            for mi in range(5):
                w = wpool.tile([128, KT, D], BF16, name=f"wt{mi}", tag=f"wt{mi}")
                for k in range(KT):
                    nc.sync.dma_start(out=w[:, k, :], in_=wmat[mi, k * 128:(k + 1) * 128, :])
                wt.append(w)
            cons = []
            for m in range(KT):
                c = wpool.tile([128, 16], F32, name=f"cons{m}", tag=f"cons{m}")
                nc.sync.dma_start(out=c, in_=consts[m * 128:(m + 1) * 128, :])
                cons.append(c)
            idt = wpool.tile([128, 128], BF16, name="idt", tag="idt")
            nc.sync.dma_start(out=idt, in_=ident[0, :, :])
            nidt = wpool.tile([128, 128], BF16, name="nidt", tag="nidt")
            nc.sync.dma_start(out=nidt, in_=ident[1, :, :])

            for ci in range(n_chunks):
                c0 = ci * nb
                ch = []
                for k in range(KT):
                    t = chpool.tile([128, 2 * nb], BF16, name=f"cha{k}", tag=f"cha{k}")
                    nc.sync.dma_start(out=t[:, H], in_=wre[k * 128:(k + 1) * 128, c0:c0 + nb])
                    nc.sync.dma_start(out=t[:, I], in_=wim[k * 128:(k + 1) * 128, c0:c0 + nb])
                    ch.append(t)
                out_t = [opool.tile([128, 2 * nb], BF16, name=f"out{m}", tag=f"out{m}")
                         for m in range(KT)]

                for d in range(DEPTH):
                    w1 = wt[0] if d == 0 else wt[1]
                    ch_next = None
                    if d < DEPTH - 1:
                        pong = "b" if d % 2 == 0 else "a"
                        ch_next = [chpool.tile([128, 2 * nb], BF16,
                                               name=f"ch{pong}{m}", tag=f"ch{pong}{m}")
                                   for m in range(KT)]

                    for m in range(KT):
                        msl = slice(m * 128, (m + 1) * 128)

                        def wave(lw, ps, stop=True):
                            # k-outer: 4 consecutive matmuls share the same
                            # stationary tile (fewer effective LDWEIGHTS).
                            for k in range(KT):
                                for j in range(2 * nb // 512):
                                    js = slice(j * 512, (j + 1) * 512)
                                    nc.tensor.matmul(ps[:, js], lw[:, k, msl],
                                                     ch[k][:, js],
                                                     start=(k == 0),
                                                     stop=(stop and k == KT - 1))

                        # Wave order h5, h2, h3, w1: the h5 z-chain (longest
                        # pointwise tail) starts 3 waves early so it finishes
                        # under the other waves; every additive term lands in
                        # `pre`, which identity-matmuls inject into the w1
                        # psum group. The per-depth serial tail is then just
                        # ident-mm + tanh + chn.
                        # NOTE: the Pool engine is banned from this pipeline -
                        # measured on hw, any concurrent Pool op demotes DVE
                        # throughput to 0.25-0.5x (SBUF port starvation).
                        ps_h5 = ppool.tile([128, 2 * nb], F32, name="psA", tag="psA")
                        wave(wt[4], ps_h5)
                        d5s = s2.tile([128, 2 * nb], BF16, name="d5s", tag="d5s")
                        nc.scalar.copy(d5s, ps_h5[:, :])
                        sq5 = s2.tile([128, 2 * nb], BF16, name="sq5", tag="sq5")
                        nc.scalar.activation(sq5, ps_h5[:, :], AF.Square)

                        # ---- h5 z-chain (DVE only), overlapped with h3/h2/w1 ----
                        r2d = s1.tile([128, nb], BF16, name="r2d", tag="r2d")
                        nc.vector.tensor_tensor(r2d, sq5[:, H], sq5[:, I], op=OP.add)
                        t5w = s1.tile([128, nb], I16, name="t5w", tag="t5w")
                        nc.vector.tensor_scalar(t5w, r2d[:, :].bitcast(U16), -2.4, K16,
                                                op0=OP.mult, op1=OP.add)
                        t5 = t5w[:, :].bitcast(BF16)
                        c2r = s1.tile([128, nb], BF16, name="c2r", tag="c2r")
                        nc.vector.tensor_tensor(c2r, sq5[:, H], sq5[:, I], op=OP.subtract)
                        e5 = s1.tile([128, nb], BF16, name="e5", tag="e5")
                        nc.vector.tensor_tensor(e5, d5s[:, H], d5s[:, I], op=OP.mult)
                        e2 = s1.tile([128, nb], BF16, name="e2", tag="e2", bufs=1)
                        nc.vector.tensor_scalar(e2, e5, 2.0, None, op0=OP.mult)
                        c22 = s1.tile([128, nb], BF16, name="c22", tag="c22", bufs=1)
                        nc.vector.tensor_scalar(c22, c2r, 2.0, None, op0=OP.mult)
                        ee4 = s1.tile([128, nb], BF16, name="ee4", tag="ee4", bufs=1)
                        nc.vector.tensor_tensor(ee4, e2, e2, op=OP.mult)
                        c4i4 = s1.tile([128, nb], BF16, name="c4i4", tag="c4i4", bufs=1)
                        nc.vector.tensor_tensor(c4i4, e2, c22, op=OP.mult)
                        sq2r5 = s1.tile([128, nb], BF16, name="sq2r5", tag="sq2r5", bufs=1)
                        nc.scalar.activation(sq2r5, c2r, AF.Square)
                        mc4r = s1.tile([128, nb], BF16, name="mc4r", tag="mc4r", bufs=1)
                        nc.vector.tensor_tensor(mc4r, ee4, sq2r5, op=OP.subtract)
                        td5 = s2.tile([128, 2 * nb], BF16, name="td5", tag="td5", bufs=1)
                        nc.vector.tensor_tensor(_as3(td5), _dup2(t5), _as3(d5s), op=OP.mult)
                        # p5hn = -(p5 real); p5i = p5 imag (mc4r = -c4r*... sign flip)
                        q1 = s1.tile([128, nb], BF16, name="q1", tag="q1", bufs=1)
                        q2 = s1.tile([128, nb], BF16, name="q2", tag="q2", bufs=1)
                        q3 = s1.tile([128, nb], BF16, name="q3", tag="q3", bufs=1)
                        q4 = s1.tile([128, nb], BF16, name="q4", tag="q4", bufs=1)
                        nc.vector.tensor_tensor(q1, mc4r, td5[:, H], op=OP.mult)
                        nc.vector.tensor_tensor(q2, c4i4, td5[:, I], op=OP.mult)
                        p5hn = s1.tile([128, nb], BF16, name="p5hn", tag="p5hn", bufs=1)
                        nc.vector.tensor_tensor(p5hn, q1, q2, op=OP.add)
                        nc.vector.tensor_tensor(q3, c4i4, td5[:, H], op=OP.mult)
                        nc.vector.tensor_tensor(q4, mc4r, td5[:, I], op=OP.mult)
                        p5i = s1.tile([128, nb], BF16, name="p5i", tag="p5i", bufs=1)
                        nc.vector.tensor_tensor(p5i, q3, q4, op=OP.subtract)

                        ps_h3 = ppool.tile([128, 2 * nb], F32, name="psB", tag="psB")
                        wave(wt[3], ps_h3)
                        d3s = s2.tile([128, 2 * nb], BF16, name="d3s", tag="d3s")
                        sq3 = s2.tile([128, 2 * nb], BF16, name="sq3", tag="sq3")
                        nc.scalar.copy(d3s, ps_h3[:, :])
                        nc.scalar.activation(sq3, ps_h3[:, :], AF.Square)
                        r2b = s1.tile([128, nb], BF16, name="r2b", tag="r2b")
                        nc.vector.tensor_tensor(r2b, sq3[:, H], sq3[:, I], op=OP.add)
                        ht = s2.tile([128, 2 * nb], BF16, name="ht", tag="ht", bufs=1)
                        nc.vector.tensor_tensor(_as3(ht), _dup2(r2b[:, :]), _as3(d3s),
                                                op=OP.mult)

                        ps_h2 = ppool.tile([128, 2 * nb], F32, name="psA", tag="psA")
                        wave(wt[2], ps_h2)
                        sq2h = s1.tile([128, nb], BF16, name="sq2h", tag="sq2h")
                        sq2i = s1.tile([128, nb], BF16, name="sq2i", tag="sq2i")
                        nc.scalar.activation(sq2h, ps_h2[:, H], AF.Square)
                        nc.scalar.activation(sq2i, ps_h2[:, I], AF.Square)
                        r2a = s1.tile([128, nb], BF16, name="r2a", tag="r2a")
                        nc.vector.tensor_tensor(r2a, sq2h, sq2i, op=OP.add)

                        ps_w1 = ppool.tile([128, 2 * nb], F32, name="psB", tag="psB")
                        wave(w1, ps_w1, stop=False)
                        # pre = ht + [r2a - p5hn | p5i], injected into the w1
                        # psum group via 4 identity matmuls.
                        rh = s1.tile([128, nb], BF16, name="rh", tag="rh", bufs=1)
                        nc.vector.tensor_tensor(rh, r2a, p5hn, op=OP.subtract)
                        pre = s2.tile([128, 2 * nb], BF16, name="pre", tag="pre")
                        nc.vector.tensor_tensor(pre[:, H], ht[:, H], rh, op=OP.add)
                        nc.vector.tensor_tensor(pre[:, I], ht[:, I], p5i, op=OP.add)
                        for j in range(2 * nb // 512):
                            js = slice(j * 512, (j + 1) * 512)
                            nc.tensor.matmul(ps_w1[:, js], idt[:, :], pre[:, js],
                                             start=False, stop=True)

                        # ---- tanh, chamber update, output accumulation ----
                        nl = s2.tile([128, 2 * nb], BF16, name="nl", tag="nl")
                        nc.scalar.activation(nl, ps_w1[:, :], AF.Tanh,
                                             scale=cons[m][:, 14:15], bias=cons[m][:, 15:16])
                        if ch_next is not None:
                            chn = ch_next[m]
                        else:
                            chn = s2.tile([128, 2 * nb], BF16, name="chl", tag="chl")
                        nc.vector.tensor_scalar(chn[:, :], nl[:, :], cons[m][:, d:d + 1],
                                                None, op0=OP.mult)
                        if d == 0:
                            nc.vector.tensor_scalar(out_t[m][:, :], chn[:, :], wout[d],
                                                    None, op0=OP.mult)
                        else:
                            wch = s2.tile([128, 2 * nb], BF16, name="wch", tag="wch", bufs=1)
                            nc.vector.tensor_scalar(wch[:, :], chn[:, :], wout[d],
                                                    None, op0=OP.mult)
                            nc.vector.tensor_tensor(out_t[m][:, :], out_t[m][:, :],
                                                    wch[:, :], op=OP.add)
                    if ch_next is not None:
                        ch = ch_next

                for m in range(KT):
                    nc.sync.dma_start(out=ore[m * 128:(m + 1) * 128, c0:c0 + nb],
                                      in_=out_t[m][:, H])
                    nc.sync.dma_start(out=oim[m * 128:(m + 1) * 128, c0:c0 + nb],
                                      in_=out_t[m][:, I])
    nc.finalize()
    return nc


def host_prep(coupling_matrix, harmonic_1, harmonic_2, harmonic_3, harmonic_5,
              mixing_scale, mixing_bias):
    damping = (0.1 / (1.0 + np.exp(np.linspace(0.0, 3.0, D)))).astype(np.float32)
    fd = np.stack([np.exp(-damping.astype(np.float64) * float(dd))
                   for dd in range(DEPTH)]).astype(np.float32)      # [7, D]
    w1_0 = (coupling_matrix + harmonic_1).astype(np.float32)
    w1_r = (w1_0 + np.eye(D, dtype=np.float32)).astype(np.float32)
    wmat = np.ascontiguousarray(np.stack([
        w1_0, w1_r, 0.5 * harmonic_2, S3 * harmonic_3, harmonic_5,
    ]).astype(ml_dtypes.bfloat16))
    consts = np.zeros((D, 16), np.float32)
    consts[:, 0:DEPTH] = fd.T
    consts[:, 14] = mixing_scale.astype(np.float32)
    consts[:, 15] = mixing_bias.astype(np.float32)
    return wmat, consts


_IDENT = np.ascontiguousarray(np.stack([np.eye(128, dtype=np.float32),
                                        -np.eye(128, dtype=np.float32)]).astype(ml_dtypes.bfloat16))

_NC_CACHE = {}


def _get_nc(n_chunks, nb):
    key = (n_chunks, nb)
    if key not in _NC_CACHE:
        _NC_CACHE[key] = build_program(n_chunks, nb)
    return _NC_CACHE[key]


def kernel(wave_real, wave_imag, coupling_matrix, harmonic_1, harmonic_2,
           harmonic_3, harmonic_5, mixing_scale, mixing_bias):
    wmat, consts = host_prep(coupling_matrix, harmonic_1, harmonic_2,
                             harmonic_3, harmonic_5, mixing_scale, mixing_bias)
    wreT = np.ascontiguousarray(
        np.asarray(wave_real, np.float32).T.astype(ml_dtypes.bfloat16))  # [D, B]
    wimT = np.ascontiguousarray(
        np.asarray(wave_imag, np.float32).T.astype(ml_dtypes.bfloat16))

    nc = _get_nc(BS // NB, NB)
    in_maps = []
    for c in range(N_CORES):
        sl = slice(c * BS, (c + 1) * BS)
        in_maps.append({
            "wre": np.ascontiguousarray(wreT[:, sl]),
            "wim": np.ascontiguousarray(wimT[:, sl]),
            "wmat": wmat,
            "ident": _IDENT,
            "consts": consts,
        })
    res = bass_utils.run_bass_kernel_spmd(nc, in_maps, core_ids=list(range(N_CORES)))
    out = np.empty((2, B, D), np.float32)
    for c in range(N_CORES):
        sl = slice(c * BS, (c + 1) * BS)
        out[0, sl, :] = res.results[c]["ore"].astype(np.float32).T
        out[1, sl, :] = res.results[c]["oim"].astype(np.float32).T
    return out
